# revision 1
# baseline (speedup 1.0000x reference)
"""GCN classifier (2x GCNConv + mean-pool + linear) on 8 Trainium2 NeuronCores.

Sharding: nodes (and their incident edges, partitioned by edge dst) are sharded
across the 8 cores; the small 128x128 weights are replicated; boundary node
features are exchanged with an AllGather of the scaled feature table after the
xw stage of each conv, before the per-edge gather/scatter.

All per-core differences are pushed into input *data* (the NEFF is SPMD: one
program for all 8 cores):
  - each core's edges are grouped into (dst-block of 128 nodes) x (src quadrant)
    cells, padded to a fixed number of 128-edge chunks (Cq) so the instruction
    stream is identical on every core
  - messages y[src] are fetched with dma_gather (int16 indices local to a src
    quadrant of 25000 rows), and scattered into PSUM with a one-hot(dst) matmul
  - degree counts / graph-id metadata are integer preprocessing done on host
"""

import math
import sys

sys.path.insert(0, "/opt/trn_rl_repo")

import ml_dtypes
import numpy as np

import concourse.bass as bass
import concourse.mybir as mybir
import concourse.tile as tile
from concourse import bacc
from concourse.bass_utils import run_bass_kernel_spmd
from concourse.masks import make_identity

BF16 = mybir.dt.bfloat16
F32 = mybir.dt.float32
I16 = mybir.dt.int16
I32 = mybir.dt.int32
NP_BF16 = ml_dtypes.bfloat16

P = 128
NCORES = 8

# problem sizes (hardcoded per the harness contract)
CFG = dict(N=100_000, E=1_600_000, G=1024, F=128, NCLS=10)

GB = 8  # dst blocks per gather group
EGB = 14  # node tiles per embedding-gather superchunk


def _plan(cfg):
    N, G = cfg["N"], cfg["G"]
    p = {}
    p["NPC"] = N // NCORES  # nodes per core
    p["NB"] = math.ceil(p["NPC"] / P)  # 128-node blocks per core
    p["NBP"] = p["NB"] * P
    p["QROWS"] = N // 4  # src quadrant rows (must be < 32768)
    assert p["QROWS"] < 32768
    p["groups"] = [
        list(range(g, min(g + GB, p["NB"]))) for g in range(0, p["NB"], GB)
    ]
    p["ESC"] = math.ceil(p["NB"] / EGB)  # embed superchunks
    p["ECOLS"] = EGB * P // 16  # idx cols per embed call
    p["NGT"] = G // P  # graph tiles
    assert G % P == 0
    return p


def _wrap16(idx_flat):
    """int16 index list -> [128, n/16] wrapped in 16 partitions, replicated 8x."""
    return np.tile(idx_flat.reshape(-1, 16).T, (8, 1))


def _prep_host(x, edge_index, batch, cfg):
    """Integer/index preprocessing + per-core metadata. Returns (per_core, Cq)."""
    pl = _plan(cfg)
    N, G = cfg["N"], cfg["G"]
    NPC, NB, NBP, QROWS = pl["NPC"], pl["NB"], pl["NBP"], pl["QROWS"]

    src = np.asarray(edge_index[0], np.int64)
    dst = np.asarray(edge_index[1], np.int64)
    batch = np.asarray(batch, np.int64)
    x = np.asarray(x, np.int64)

    deg_p1 = (np.bincount(dst, minlength=N) + 1).astype(np.float32)
    cnt = np.maximum(np.bincount(batch, minlength=G), 1).astype(np.float32)
    cnt_pt = cnt.reshape(pl["NGT"], P).T.copy()  # [P, NGT]

    core_of = dst // NPC
    per_core_edge = []  # (sorted sloc, sorted w, counts per cell)
    Cq = 1
    for k in range(NCORES):
        m = core_of == k
        s_k, d_k = src[m], dst[m] - k * NPC
        blk = d_k >> 7
        q = s_k // QROWS
        key = (blk * 4 + q).astype(np.int64)
        order = np.argsort(key, kind="stable")
        counts = np.bincount(key, minlength=NB * 4)
        Cq = max(Cq, math.ceil(counts.max() / P))
        sloc = (s_k - q * QROWS).astype(np.int16)[order]
        w = (d_k & 127).astype(np.float32)[order]
        per_core_edge.append((sloc, w, counts))

    per_core = []
    for k in range(NCORES):
        sloc, w, counts = per_core_edge[k]
        cap = Cq * P
        src_pad = np.zeros((NB * 4, cap), np.int16)
        dst_pad = np.full((NB * 4, cap), -1.0, np.float32)
        starts = np.concatenate([[0], np.cumsum(counts)])
        for cell in range(NB * 4):
            c0, c1 = starts[cell], starts[cell + 1]
            n = c1 - c0
            if n:
                src_pad[cell, :n] = sloc[c0:c1]
                dst_pad[cell, :n] = w[c0:c1]

        idx_cols, dst_cols = [], []
        for blocks in pl["groups"]:
            for q in range(4):
                cells = [b * 4 + q for b in blocks]
                flat = src_pad[cells].reshape(-1)
                idx_cols.append(_wrap16(flat))
            for b in blocks:
                # block-major: the 4*Cq chunk columns of block b, (q, cc) order
                cells = [b * 4 + q for q in range(4)]
                dst_cols.append(dst_pad[cells].reshape(-1, P).T)
        edge_idx = np.concatenate(idx_cols, 1)  # [128, TOTCOL] i16
        dstc = np.concatenate(dst_cols, 1).astype(NP_BF16)  # [128, NCH]

        # degree (layout [p, c] = local node c*128+p), pad nodes -> deg+1 = 1
        dp = np.ones(NBP, np.float32)
        dp[:NPC] = deg_p1[k * NPC : (k + 1) * NPC]
        dp = dp.reshape(NB, P).T.copy()

        # pool metadata
        bl = batch[k * NPC : (k + 1) * NPC]
        gbase = int(bl[0])
        gspan = int(bl[-1]) - gbase + 1
        assert gspan <= 2 * P, f"core {k} graph span {gspan} > 256"
        blf = np.full(NBP, -1.0, np.float32)
        blf[:NPC] = (bl - gbase).astype(np.float32)
        bl0 = blf.reshape(NB, P).T.astype(np.float32)
        bl1 = (blf - P).reshape(NB, P).T.astype(np.float32)
        gidx = np.zeros((P, 2), np.int32)
        for h in range(2):
            v = gbase + h * P + np.arange(P)
            v = np.where(v < G, v, G + (v % 8))
            gidx[:, h] = v

        # embedding gather indices (x values < 256 fit int16)
        xi = np.zeros((NBP, 3), np.int16)
        xi[:NPC] = x[k * NPC : (k + 1) * NPC].astype(np.int16)
        ecols = []
        for j in range(3):
            for s in range(pl["ESC"]):
                seg = np.zeros(EGB * P, np.int16)
                src_seg = xi[s * EGB * P : (s + 1) * EGB * P, j]
                seg[: len(src_seg)] = src_seg
                ecols.append(_wrap16(seg))
        emb_idx = np.concatenate(ecols, 1)

        per_core.append(
            dict(
                deg_p1=dp,
                bl0=bl0,
                bl1=bl1,
                gidx=gidx,
                cnt=cnt_pt,
                emb_idx=emb_idx,
                edge_idx=edge_idx,
                dst_cols=dstc,
            )
        )
    return per_core, Cq, pl


def _build(cfg, Cq, pl, totcol, nch, necol):
    """Build the SPMD Bass program (one NEFF for all 8 cores)."""
    import os
    PHASES = int(os.environ.get("K_PHASES", "9"))  # 1=embed 2=+conv1 3=+conv2 9=all
    SUB = int(os.environ.get("K_SUB", "9"))  # 1=xw+AG 2=+gathers 3=+onehot 4=+matmul/epi
    NOAG = int(os.environ.get("K_NOAG", "0"))  # 1: replace AllGather with local copies
    N, G, F, NCLS = cfg["N"], cfg["G"], cfg["F"], cfg["NCLS"]
    NPC, NB, QROWS, NGT = pl["NPC"], pl["NB"], pl["QROWS"], pl["NGT"]
    groups, ESC, ECOLS = pl["groups"], pl["ESC"], pl["ECOLS"]

    nc = bacc.Bacc("TRN2", num_devices=NCORES, num_swdge_queues=4)
    RG = [list(range(NCORES))]

    # ---- I/O ----
    tabs = [
        nc.dram_tensor("shape_tab", [16, F], F32, kind="ExternalInput"),
        nc.dram_tensor("color_tab", [16, F], F32, kind="ExternalInput"),
        nc.dram_tensor("pos_tab", [256, F], F32, kind="ExternalInput"),
    ]
    W1d = nc.dram_tensor("W1", [F, F], F32, kind="ExternalInput")
    W2d = nc.dram_tensor("W2", [F, F], F32, kind="ExternalInput")
    b1d = nc.dram_tensor("b1", [1, F], F32, kind="ExternalInput")
    b2d = nc.dram_tensor("b2", [1, F], F32, kind="ExternalInput")
    Wld = nc.dram_tensor("Wlin", [F, NCLS], F32, kind="ExternalInput")
    bld = nc.dram_tensor("blin", [1, NCLS], F32, kind="ExternalInput")
    degd = nc.dram_tensor("deg_p1", [P, NB], F32, kind="ExternalInput")
    bl0d = nc.dram_tensor("bl0", [P, NB], F32, kind="ExternalInput")
    bl1d = nc.dram_tensor("bl1", [P, NB], F32, kind="ExternalInput")
    gixd = nc.dram_tensor("gidx", [P, 2], I32, kind="ExternalInput")
    cntd = nc.dram_tensor("cnt", [P, NGT], F32, kind="ExternalInput")
    eixd = nc.dram_tensor("emb_idx", [P, necol], I16, kind="ExternalInput")
    xixd = nc.dram_tensor("edge_idx", [P, totcol], I16, kind="ExternalInput")
    dcd = nc.dram_tensor("dst_cols", [P, nch], BF16, kind="ExternalInput")
    outd = nc.dram_tensor("out", [G, NCLS], F32, kind="ExternalOutput")

    with tile.TileContext(nc) as tc:
        import contextlib

        ctx = contextlib.ExitStack()
        persist = ctx.enter_context(tc.tile_pool(name="persist", bufs=1))
        dramp = ctx.enter_context(tc.tile_pool(name="dramp", bufs=1, space="DRAM"))
        tp_pool = ctx.enter_context(tc.tile_pool(name="tp", bufs=2, space="PSUM"))
        xw_pool = ctx.enter_context(tc.tile_pool(name="xw", bufs=2, space="PSUM"))
        acc_pool = ctx.enter_context(tc.tile_pool(name="acc", bufs=2, space="PSUM"))
        pacc_pool = ctx.enter_context(tc.tile_pool(name="pacc", bufs=1, space="PSUM"))
        sb_pool = ctx.enter_context(tc.tile_pool(name="work", bufs=3))
        msg_pool = ctx.enter_context(tc.tile_pool(name="msg", bufs=5))
        oh_pool = ctx.enter_context(tc.tile_pool(name="oh", bufs=3))
        ix_pool = ctx.enter_context(tc.tile_pool(name="ix", bufs=4))
        craw = ctx.enter_context(tc.tile_pool(name="craw", bufs=1))

        def T(shape, dt, space=None, addr_space="Local", name=None):
            pool = dramp if space == "DRAM" else persist
            return pool.tile(shape, dt, tag=name, name=name, addr_space=addr_space)

        # ---- internal DRAM ----
        y_slice = [
            T([NPC, F], BF16, space="DRAM", name=f"y_slice{c}") for c in range(2)
        ]
        y_full = [
            T([NCORES * NPC, F], BF16, space="DRAM",
              addr_space="Local" if NOAG else "Shared", name=f"y_full{c}")
            for c in range(2)
        ]
        dram_sums = T([G + 8, F], F32, space="DRAM", name="dram_sums")
        ar_sums = T([G + 8, F], F32, space="DRAM", addr_space="Shared",
                          name="ar_sums")

        # ---- persistent SBUF ----
        hA = T([P, NB * F], BF16, name="hA")
        hB = T([P, NB * F], BF16, name="hB")
        y_nm = T([P, NB * F], BF16, name="y_nm")
        dstc_sb = T([P, nch], BF16, name="dstc_sb")
        nc.sync.dma_start(out=dstc_sb[:], in_=dcd[:])

        # constants
        iota_i = craw.tile([P, P], I32, tag="iota_i", name="iota_i")
        nc.gpsimd.iota(iota_i[:], pattern=[[1, P]], base=0, channel_multiplier=0)
        iota_bf = T([P, P], BF16, name="iota_bf")
        nc.vector.tensor_copy(iota_bf[:], iota_i[:])
        iota_f = T([P, P], F32, name="iota_f")
        nc.vector.tensor_copy(iota_f[:], iota_i[:])
        id_f32 = T([P, P], F32, name="id_f32")
        make_identity(nc, id_f32[:])
        id_bf = T([P, P], BF16, name="id_bf")
        nc.vector.tensor_copy(id_bf[:], id_f32[:])
        ones_row = T([1, P], F32, name="ones_row")
        nc.vector.memset(ones_row[:], 1.0)

        def load_cast(name, dram, shape, dt_in, dt_out):
            t = T(shape, dt_out, name=name)
            if dt_out == dt_in:
                nc.sync.dma_start(out=t[:], in_=dram[:])
            else:
                # NB: SWDGE cast-DMA + indirect_dma in one program crashes the
                # device (observed NRT_EXEC_UNIT_UNRECOVERABLE) - cast on DVE.
                raw = craw.tile(shape, dt_in, tag=name + "_r", name=name + "_r")
                nc.sync.dma_start(out=raw[:], in_=dram[:])
                nc.vector.tensor_copy(t[:], raw[:])
            return t

        Wc = [
            load_cast("W1", W1d, [F, F], F32, BF16),
            load_cast("W2", W2d, [F, F], F32, BF16),
        ]
        bc = [
            load_cast("b1", b1d, [1, F], F32, F32),
            load_cast("b2", b2d, [1, F], F32, F32),
        ]
        Wl_sb = load_cast("Wl", Wld, [F, NCLS], F32, F32)
        bl_sb = load_cast("bl", bld, [1, NCLS], F32, F32)
        bl0_sb = load_cast("bl0", bl0d, [P, NB], F32, F32)
        bl1_sb = load_cast("bl1", bl1d, [P, NB], F32, F32)
        cnt_sb = load_cast("cnt", cntd, [P, NGT], F32, F32)
        gix_sb = load_cast("gix", gixd, [P, 2], I32, I32)
        eix_sb = load_cast("eix", eixd, [P, necol], I16, I16)

        # dinv = 1/sqrt(deg+1); rdinv = sqrt(deg+1) (transposed for bias matmul)
        deg_sb = craw.tile([P, NB], F32, tag="deg_sb", name="deg_sb")
        nc.sync.dma_start(out=deg_sb[:], in_=degd[:])
        sq_sb = T([P, NB], F32, name="sq_sb")
        nc.scalar.sqrt(sq_sb[:], deg_sb[:])
        dinv = T([P, NB], F32, name="dinv")
        nc.vector.reciprocal(dinv[:], sq_sb[:])


        # zero dram_sums (pool scatter target) early
        zsb = craw.tile([P, 512], F32, tag="zsb", name="zsb")
        nc.vector.memset(zsb[:], 0.0)
        nrow = G + 8
        r = 0
        while r < nrow:
            take = min(512, ((nrow - r) // P) * P)
            pp = P
            if take == 0:
                take = nrow - r
                pp = take
            nc.sync.dma_start(
                out=dram_sums[r : r + take, :].rearrange("(c p) f -> p c f", p=pp),
                in_=zsb[:pp, : take * F // pp].rearrange("p (c f) -> p c f", f=F),
            )
            r += take

        # ---------------- embedding ----------------
        for s in range(ESC):
            t0 = s * EGB
            t1 = min(t0 + EGB, NB)
            nt = t1 - t0
            nidx = nt * P
            g_tiles = []
            for j in range(3):
                g = msg_pool.tile([P, EGB, F], F32, tag="msg")
                eoff = (j * ESC + s) * ECOLS
                nc.gpsimd.dma_gather(
                    out_ap=g[:, :nt, :],
                    in_ap=tabs[j][:, :],
                    idxs_ap=eix_sb[:, eoff : eoff + nidx // 16],
                    num_idxs=nidx,
                    num_idxs_reg=nidx,
                    elem_size=F,
                    single_packet=False,
                    queue_num=j,
                )
                g_tiles.append(g)
            nc.vector.tensor_tensor(
                out=g_tiles[0][:, :nt, :],
                in0=g_tiles[0][:, :nt, :],
                in1=g_tiles[1][:, :nt, :],
                op=mybir.AluOpType.add,
            )
            nc.vector.tensor_tensor(
                out=hA[:, t0 * F : t1 * F].rearrange("p (c f) -> p c f", f=F),
                in0=g_tiles[0][:, :nt, :],
                in1=g_tiles[2][:, :nt, :],
                op=mybir.AluOpType.add,
            )

        # ---------------- two GCN convs ----------------
        for conv in range(min(2, max(0, PHASES - 1))):
            hin = hA if conv == 0 else hB
            hout = hB if conv == 0 else hA

            # b_bcast[n, f] = b[f] replicated down partitions (rank-1 via PE)
            bb_ps = xw_pool.tile([P, P], F32, tag="xw")
            nc.tensor.matmul(bb_ps[:], lhsT=ones_row[:], rhs=bc[conv][:],
                             start=True, stop=True)
            b_bcast = craw.tile([P, P], F32, tag=f"b_bcast{conv}", name=f"b_bcast{conv}")
            nc.scalar.copy(b_bcast[:], bb_ps[:])

            # xw phase: y = dinv * (h @ W) -> y_nm (SBUF) and y_slice (DRAM)
            for t in range(NB):
                tp = tp_pool.tile([P, P], BF16, tag="tp")
                nc.tensor.transpose(tp[:], hin[:, t * F : (t + 1) * F], id_bf[:])
                hT = sb_pool.tile([P, P], BF16, tag="hT")
                nc.scalar.copy(hT[:], tp[:])
                xw = xw_pool.tile([P, P], F32, tag="xw")
                nc.tensor.matmul(xw[:], lhsT=hT[:], rhs=Wc[conv][:], start=True, stop=True)
                nc.scalar.activation(
                    y_nm[:, t * F : (t + 1) * F],
                    xw[:],
                    mybir.ActivationFunctionType.Copy,
                    scale=dinv[:, t : t + 1],
                )
            # y_nm -> y_slice DRAM (node-major rows)
            full_b = NPC // P
            nc.sync.dma_start(
                out=y_slice[conv][: full_b * P, :].rearrange("(c p) f -> p c f", p=P),
                in_=y_nm[:, : full_b * F].rearrange("p (c f) -> p c f", f=F),
            )
            rem = NPC - full_b * P
            if rem:
                nc.sync.dma_start(
                    out=y_slice[conv][full_b * P :, :],
                    in_=y_nm[:rem, full_b * F : (full_b + 1) * F],
                )
            if NOAG:
                for kk in range(NCORES):
                    nc.sync.dma_start(
                        out=y_full[conv][kk * NPC : (kk + 1) * NPC, :],
                        in_=y_slice[conv][:],
                    )
            else:
                nc.gpsimd.collective_compute(
                    "AllGather",
                    mybir.AluOpType.bypass,
                    replica_groups=RG,
                    ins=[y_slice[conv][:]],
                    outs=[y_full[conv][:]],
                )

            # scatter phase
            ch_off = 0  # chunk offset into dst_cols
            col_off = 0  # column offset into edge_idx
            for blocks in (groups if SUB >= 2 else []):
                nblk = len(blocks)
                nch_q = nblk * Cq
                nidx = nch_q * P
                msgs = []
                for q in range(4):
                    ixt = ix_pool.tile([P, GB * Cq * 8], I16, tag="ix")
                    nc.sync.dma_start(
                        out=ixt[:, : nidx // 16],
                        in_=xixd[:, col_off : col_off + nidx // 16],
                    )
                    col_off += nidx // 16
                    msg = msg_pool.tile([P, GB * Cq, F], BF16, tag="msg")
                    nc.gpsimd.dma_gather(
                        out_ap=msg[:, :nch_q, :],
                        in_ap=y_full[conv][q * QROWS : (q + 1) * QROWS, :],
                        idxs_ap=ixt[:, : nidx // 16],
                        num_idxs=nidx,
                        num_idxs_reg=nidx,
                        elem_size=F,
                        single_packet=False,
                        queue_num=q,
                    )
                    msgs.append(msg)
                if SUB < 3:
                    ch_off += 4 * nblk * Cq
                    continue
                for bi, b in enumerate(blocks):
                    oh = oh_pool.tile([P, 4 * Cq, P], BF16, tag="oh")
                    nc.vector.tensor_tensor(
                        out=oh[:],
                        in0=iota_bf[:].unsqueeze(1).broadcast_to([P, 4 * Cq, P]),
                        in1=dstc_sb[:, ch_off + bi * 4 * Cq : ch_off + (bi + 1) * 4 * Cq]
                        .unsqueeze(2)
                        .broadcast_to([P, 4 * Cq, P]),
                        op=mybir.AluOpType.is_equal,
                    )
                    if SUB < 4:
                        continue
                    acc = acc_pool.tile([P, P], F32, tag="acc")
                    j = 0
                    for q in range(4):
                        for cc in range(Cq):
                            nc.tensor.matmul(
                                acc[:],
                                lhsT=oh[:, q * Cq + cc, :],
                                rhs=msgs[q][:, bi * Cq + cc, :],
                                start=(j == 0),
                                stop=(j == 4 * Cq - 1),
                            )
                            j += 1
                    # bias: bb = b (x) rdinv (cancels the later *dinv); ACT op
                    bb = sb_pool.tile([P, P], F32, tag="bb")
                    nc.scalar.activation(
                        bb[:], b_bcast[:], mybir.ActivationFunctionType.Copy,
                        scale=sq_sb[:, b : b + 1],
                    )
                    hs = sb_pool.tile([P, P], F32, tag="ep")
                    nc.vector.tensor_tensor(
                        out=hs[:],
                        in0=acc[:],
                        in1=y_nm[:, b * F : (b + 1) * F],
                        op=mybir.AluOpType.add,
                    )
                    nc.vector.tensor_tensor(
                        out=hs[:], in0=hs[:], in1=bb[:], op=mybir.AluOpType.add,
                    )
                    nc.vector.tensor_tensor(
                        out=hs[:],
                        in0=hs[:],
                        in1=dinv[:, b : b + 1].to_broadcast([P, P]),
                        op=mybir.AluOpType.mult,
                    )
                    nc.scalar.activation(
                        hout[:, b * F : (b + 1) * F],
                        hs[:],
                        mybir.ActivationFunctionType.Relu,
                    )
                ch_off += 4 * nch_q

        # ---------------- global mean pool + linear ----------------
        do_pool = PHASES >= 9
        if do_pool:
            pacc = [pacc_pool.tile([P, P], F32, tag=f"pacc{h}", name=f"pacc{h}") for h in range(2)]
            bls = [bl0_sb, bl1_sb]
            for t in range(NB):
                for h in range(2):
                    oht = sb_pool.tile([P, P], BF16, tag="pooloh")
                    nc.vector.tensor_tensor(
                        out=oht[:],
                        in0=iota_f[:],
                        in1=bls[h][:, t : t + 1].to_broadcast([P, P]),
                        op=mybir.AluOpType.is_equal,
                    )
                    nc.tensor.matmul(
                        pacc[h][:],
                        lhsT=oht[:],
                        rhs=hA[:, t * F : (t + 1) * F],
                        start=(t == 0),
                        stop=(t == NB - 1),
                    )
            for h in range(2):
                se = sb_pool.tile([P, P], F32, tag="ep")
                nc.vector.tensor_copy(se[:], pacc[h][:])
                nc.gpsimd.indirect_dma_start(
                    out=dram_sums[:],
                    out_offset=bass.IndirectOffsetOnAxis(ap=gix_sb[:, h : h + 1], axis=0),
                    in_=se[:],
                    in_offset=None,
                )
            nc.gpsimd.collective_compute(
                "AllReduce",
                mybir.AluOpType.add,
                replica_groups=RG,
                ins=[dram_sums[:]],
                outs=[ar_sums[:]],
            )
            recip = T([P, NGT], F32, name="recip")
            nc.vector.reciprocal(recip[:], cnt_sb[:])
            for t in range(NGT):
                art = sb_pool.tile([P, P], F32, tag="art")
                nc.sync.dma_start(out=art[:], in_=ar_sums[t * P : (t + 1) * P, :])
                pooled = sb_pool.tile([P, P], F32, tag="ep")
                nc.vector.tensor_tensor(
                    out=pooled[:],
                    in0=art[:],
                    in1=recip[:, t : t + 1].to_broadcast([P, P]),
                    op=mybir.AluOpType.mult,
                )
                ptp = tp_pool.tile([P, P], F32, tag="tp")
                nc.tensor.transpose(ptp[:], pooled[:], id_f32[:])
                pooledT = sb_pool.tile([P, P], F32, tag="hT")
                nc.scalar.copy(pooledT[:], ptp[:])
                op = xw_pool.tile([P, NCLS], F32, tag="xw")
                nc.tensor.matmul(op[:], lhsT=pooledT[:], rhs=Wl_sb[:], start=True, stop=False)
                nc.tensor.matmul(op[:], lhsT=ones_row[:], rhs=bl_sb[:], start=False, stop=True)
                oute = sb_pool.tile([P, NCLS], F32, tag="oute")
                nc.vector.tensor_copy(oute[:], op[:])
                nc.sync.dma_start(out=outd[t * P : (t + 1) * P, :], in_=oute[:])


        else:
            dummy = sb_pool.tile([P, NCLS], F32, tag="oute", name="dummy")
            nc.vector.memset(dummy[:], 0.0)
            for t in range(NGT):
                nc.sync.dma_start(out=outd[t * P : (t + 1) * P, :], in_=dummy[:])

        ctx.close()
    nc.compile()
    return nc


_CACHE = {}


def _get_nc(cfg, Cq, pl, totcol, nch, necol):
    key = (tuple(sorted(cfg.items())), Cq, totcol, nch, necol)
    if key not in _CACHE:
        _CACHE[key] = _build(cfg, Cq, pl, totcol, nch, necol)
    return _CACHE[key]


def run(inputs, cfg, trace=False):
    x = np.asarray(inputs["x"])
    per_core, Cq, pl = _prep_host(x, np.asarray(inputs["edge_index"]),
                                  np.asarray(inputs["batch"]), cfg)
    totcol = per_core[0]["edge_idx"].shape[1]
    nch = per_core[0]["dst_cols"].shape[1]
    necol = per_core[0]["emb_idx"].shape[1]
    nc = _get_nc(cfg, Cq, pl, totcol, nch, necol)

    shared = dict(
        shape_tab=np.asarray(inputs["shape_tab"], np.float32),
        color_tab=np.asarray(inputs["color_tab"], np.float32),
        pos_tab=np.asarray(inputs["pos_tab"], np.float32),
        W1=np.asarray(inputs["W1"], np.float32),
        W2=np.asarray(inputs["W2"], np.float32),
        b1=np.asarray(inputs["b1"], np.float32).reshape(1, -1),
        b2=np.asarray(inputs["b2"], np.float32).reshape(1, -1),
        Wlin=np.asarray(inputs["Wlin"], np.float32),
        blin=np.asarray(inputs["blin"], np.float32).reshape(1, -1),
    )
    in_maps = [{**shared, **per_core[k]} for k in range(NCORES)]
    res = run_bass_kernel_spmd(nc, in_maps, list(range(NCORES)), trace=trace)
    out = np.asarray(res.results[0]["out"], np.float32)
    return out, res


def kernel(**inputs) -> np.ndarray:
    out, _ = run(inputs, CFG)
    return out



# revision 21
# speedup vs baseline: 1.0497x; 1.0497x over previous
"""GCN classifier (2x GCNConv + mean-pool + linear) on 8 Trainium2 NeuronCores.

v2: gather-descriptor-roofline design.
  - nodes (and incident edges, partitioned by dst) sharded over 8 cores;
    128x128 weights replicated; SPMD single NEFF.
  - per-edge messages fetched with dma_gather from a per-region (quarter
    of the node space, <32768 rows for int16 idx) AllGather'd feature
    table; 4 SWDGE queues (one per region) kept saturated.
  - scatter to dst via one-hot matmuls whose one-hot matrices are built
    on HOST (values = dinv[src]*dinv[dst], bias slot = row of ones that
    multiplies a `b` row appended to the table) and streamed from DRAM
    with plain HWDGE DMA. Self-loop added with an identity matmul; the
    whole conv epilogue is one Relu activation.
  - embedding is folded into conv1's xw: xw1 = sum_j OH_j @ (tab_j@W1),
    with OH_j built on-device from x values (no dma_gather).
  - AllGather sliced into 4 region collectives, pipelined with compute.
  - mean-pool accumulation fused into conv2's epilogue.
"""

import math
import sys

sys.path.insert(0, "/opt/trn_rl_repo")

import ml_dtypes
import numpy as np

import concourse.bass as bass
import concourse.mybir as mybir
import concourse.tile as tile
from concourse import bacc
from concourse.bass_utils import run_bass_kernel_spmd
from concourse.masks import make_identity

BF16 = mybir.dt.bfloat16
F32 = mybir.dt.float32
I16 = mybir.dt.int16
I32 = mybir.dt.int32
NP_BF16 = ml_dtypes.bfloat16

P = 128
NCORES = 8

CFG = dict(N=100_000, E=1_600_000, G=1024, F=128, NCLS=10)

GB = 8  # dst blocks per gather group
RSIZE = 3200  # region rows per core slice (25 blocks; last region 23 blocks)
NREG = 4


def _plan(cfg):
    N, G = cfg["N"], cfg["G"]
    p = {}
    p["NPC"] = N // NCORES  # 12500 nodes per core
    p["NB"] = math.ceil(p["NPC"] / P)  # 98 blocks
    p["NBP"] = p["NB"] * P  # 12544
    # region r covers blocks [25r, 25r+25) (last: 23); rows [3200r, ...)
    p["reg_blocks"] = [(25 * r, min(25 * r + 25, p["NB"])) for r in range(NREG)]
    p["TROWS"] = NCORES * RSIZE  # 25600 rows per region table
    assert p["TROWS"] < 32768
    # region 3 only has 2944 data rows per core; row 2944 of core 0's slice
    # (= table row 2944 of region 3) carries the conv bias vector.
    p["BIAS_ROW"] = 23 * P
    p["groups"] = [
        list(range(g, min(g + GB, p["NB"]))) for g in range(0, p["NB"], GB)
    ]
    p["NGT"] = G // P
    assert G % P == 0
    return p


def _wrap16(idx_flat):
    """int16 index list -> [128, n/16] wrapped in 16 partitions, replicated 8x."""
    return np.tile(idx_flat.reshape(-1, 16).T, (8, 1))


def _prep_host(x, edge_index, batch, cfg):
    """Integer/index preprocessing + per-core metadata. Returns (per_core, Cq, pl)."""
    pl = _plan(cfg)
    N, G = cfg["N"], cfg["G"]
    NPC, NB, NBP = pl["NPC"], pl["NB"], pl["NBP"]

    src = np.asarray(edge_index[0], np.int64)
    dst = np.asarray(edge_index[1], np.int64)
    batch = np.asarray(batch, np.int64)
    x = np.asarray(x, np.int64)

    deg_p1 = (np.bincount(dst, minlength=N) + 1).astype(np.float64)
    dinv = 1.0 / np.sqrt(deg_p1)  # global normalization coefs
    cnt = np.maximum(np.bincount(batch, minlength=G), 1).astype(np.float32)
    cnt_pt = cnt.reshape(pl["NGT"], P).T.copy()  # [P, NGT]

    # region-local row index of a global src node
    s_core = src // NPC
    s_off = src % NPC
    s_reg = np.minimum(s_off // RSIZE, NREG - 1)
    s_row = s_core * RSIZE + (s_off - s_reg * RSIZE)  # < 25600

    core_of = dst // NPC
    per_core_edge = []
    Cq = 1
    for k in range(NCORES):
        m = core_of == k
        d_k = dst[m] - k * NPC
        blk = d_k >> 7
        key = (blk * NREG + s_reg[m]).astype(np.int64)
        order = np.argsort(key, kind="stable")
        counts = np.bincount(key, minlength=NB * NREG)
        Cq = max(Cq, math.ceil(counts.max() / P))
        sloc = s_row[m].astype(np.int16)[order]
        dslot = (d_k & 127).astype(np.int16)[order]
        coef = (dinv[src[m]] * dinv[dst[m]]).astype(np.float32)[order]
        per_core_edge.append((sloc, dslot, coef, counts))

    cap = Cq * P
    per_core = []
    for k in range(NCORES):
        sloc, dslot, coef, counts = per_core_edge[k]
        src_pad = np.zeros((NB * NREG, cap), np.int16)
        dst_pad = np.full((NB * NREG, cap), -1, np.int16)
        coef_pad = np.zeros((NB * NREG, cap), np.float32)
        starts = np.concatenate([[0], np.cumsum(counts)])
        for cell in range(NB * NREG):
            c0, c1 = starts[cell], starts[cell + 1]
            n = c1 - c0
            if n:
                src_pad[cell, :n] = sloc[c0:c1]
                dst_pad[cell, :n] = dslot[c0:c1]
                coef_pad[cell, :n] = coef[c0:c1]

        # bias slot: one pad slot per block (in its region-3 cell, whose table
        # row BIAS_ROW holds b) -> gathers the b row; one-hot row = all ones
        # so every dst of the block receives +b[f].
        BIAS = pl["BIAS_ROW"]
        for b in range(NB):
            cell = b * NREG + (NREG - 1)
            n = counts[cell]
            assert n < cap, f"core {k} block {b}: no pad slot for bias"
            src_pad[cell, n] = BIAS
            dst_pad[cell, n] = -2  # marker: full row of ones

        # gather index stream per (group, region): concat cell streams
        idx_cols = []
        for blocks in pl["groups"]:
            for r in range(NREG):
                cells = [b * NREG + r for b in blocks]
                flat = src_pad[cells].reshape(-1)
                idx_cols.append(_wrap16(flat))
        edge_idx = np.concatenate(idx_cols, 1)  # [128, totcol] i16

        # host-built one-hot matrices, block-major, chunk order (r, cc):
        # oh[p, b*4Cq*128 + (r*Cq+cc)*128 + i] = coef of edge slot p of that
        # chunk if its dst slot == i (or 1.0 for the bias slot's full row).
        dst3 = dst_pad.reshape(NB, NREG * Cq, P)  # [b, chunk, p]
        coef3 = coef_pad.reshape(NB, NREG * Cq, P)
        iota = np.arange(P, dtype=np.int16)
        onehot = (dst3[:, :, :, None] == iota[None, None, None, :]).astype(
            np.float32
        )
        onehot *= coef3[:, :, :, None]
        onehot += (dst3[:, :, :, None] == -2).astype(np.float32)
        # -> [p, b, chunk, i] -> [128, NB*4Cq*128]
        oh = (
            onehot.transpose(2, 0, 1, 3)
            .reshape(P, NB * NREG * Cq * P)
            .astype(NP_BF16)
        )

        # degree (layout [p, c] = local node c*128+p), pad nodes -> deg+1 = 1
        dp = np.ones(NBP, np.float32)
        dp[: NPC] = deg_p1[k * NPC : (k + 1) * NPC].astype(np.float32)
        dp = dp.reshape(NB, P).T.copy()

        # pool metadata
        bl = batch[k * NPC : (k + 1) * NPC]
        gbase = int(bl[0])
        gspan = int(bl[-1]) - gbase + 1
        assert gspan <= 2 * P, f"core {k} graph span {gspan} > 256"
        blf = np.full(NBP, -1.0, np.float32)
        blf[: NPC] = (bl - gbase).astype(np.float32)
        bl0 = blf.reshape(NB, P).T.astype(np.float32)
        bl1 = (blf - P).reshape(NB, P).T.astype(np.float32)
        gidx = np.zeros((P, 2), np.int32)
        for h in range(2):
            v = gbase + h * P + np.arange(P)
            v = np.where(v < G, v, G + (v % 8))
            gidx[:, h] = v

        # x values as columns: xcat[p, t*4+j] = x_j[node t*128+p] (j=3 dups x2
        # for the pos table's second 128-row chunk)
        xi = np.zeros((NBP, 3), np.int16)
        xi[: NPC] = x[k * NPC : (k + 1) * NPC].astype(np.int16)
        x3 = xi.reshape(NB, P, 3)
        xcat = np.zeros((P, NB * 4), NP_BF16)
        for j in range(3):
            xcat[:, j::4] = x3[:, :, j].T
        xcat[:, 3::4] = x3[:, :, 2].T

        per_core.append(
            dict(
                deg_p1=dp,
                bl0=bl0,
                bl1=bl1,
                gidx=gidx,
                cnt=cnt_pt,
                xvals=xcat,
                edge_idx=edge_idx,
                oh=oh,
            )
        )
    return per_core, Cq, pl


def _build(cfg, Cq, pl, totcol, nohc):
    """Build the SPMD Bass program (one NEFF for all 8 cores)."""
    N, G, F, NCLS = cfg["N"], cfg["G"], cfg["F"], cfg["NCLS"]
    NPC, NB, NGT = pl["NPC"], pl["NB"], pl["NGT"]
    groups, reg_blocks, TROWS = pl["groups"], pl["reg_blocks"], pl["TROWS"]
    CPB = NREG * Cq  # chunks per block

    nc = bacc.Bacc("TRN2", num_devices=NCORES, num_swdge_queues=4)
    RG = [list(range(NCORES))]

    # ---- I/O ----
    tabs = [
        nc.dram_tensor("shape_tab", [16, F], F32, kind="ExternalInput"),
        nc.dram_tensor("color_tab", [16, F], F32, kind="ExternalInput"),
        nc.dram_tensor("pos_tab", [256, F], F32, kind="ExternalInput"),
    ]
    W1d = nc.dram_tensor("W1", [F, F], F32, kind="ExternalInput")
    W2d = nc.dram_tensor("W2", [F, F], F32, kind="ExternalInput")
    b1d = nc.dram_tensor("b1", [1, F], F32, kind="ExternalInput")
    b2d = nc.dram_tensor("b2", [1, F], F32, kind="ExternalInput")
    Wld = nc.dram_tensor("Wlin", [F, NCLS], F32, kind="ExternalInput")
    bld = nc.dram_tensor("blin", [1, NCLS], F32, kind="ExternalInput")
    degd = nc.dram_tensor("deg_p1", [P, NB], F32, kind="ExternalInput")
    bl0d = nc.dram_tensor("bl0", [P, NB], F32, kind="ExternalInput")
    bl1d = nc.dram_tensor("bl1", [P, NB], F32, kind="ExternalInput")
    gixd = nc.dram_tensor("gidx", [P, 2], I32, kind="ExternalInput")
    cntd = nc.dram_tensor("cnt", [P, NGT], F32, kind="ExternalInput")
    xvd = nc.dram_tensor("xvals", [P, NB * 4], BF16, kind="ExternalInput")
    xixd = nc.dram_tensor("edge_idx", [P, totcol], I16, kind="ExternalInput")
    ohd = nc.dram_tensor("oh", [P, nohc], BF16, kind="ExternalInput")
    outd = nc.dram_tensor("out", [G, NCLS], F32, kind="ExternalOutput")

    with tile.TileContext(nc) as tc:
        import contextlib

        ctx = contextlib.ExitStack()
        persist = ctx.enter_context(tc.tile_pool(name="persist", bufs=1))
        dramp = ctx.enter_context(tc.tile_pool(name="dramp", bufs=1, space="DRAM"))
        etp_pool = ctx.enter_context(tc.tile_pool(name="etp", bufs=1, space="PSUM"))
        xw_pool = ctx.enter_context(tc.tile_pool(name="xw", bufs=2, space="PSUM"))
        tp_pool = ctx.enter_context(tc.tile_pool(name="tp", bufs=1, space="PSUM"))
        acc_pool = ctx.enter_context(tc.tile_pool(name="acc", bufs=2, space="PSUM"))
        pacc_pool = ctx.enter_context(tc.tile_pool(name="pacc", bufs=1, space="PSUM"))
        sb_pool = ctx.enter_context(tc.tile_pool(name="work", bufs=4))
        yreg_pool = ctx.enter_context(tc.tile_pool(name="yreg", bufs=2))
        msg_pool = ctx.enter_context(tc.tile_pool(name="msg", bufs=8))
        oh_pool = ctx.enter_context(tc.tile_pool(name="oh", bufs=4))
        ix_pool = ctx.enter_context(tc.tile_pool(name="ix", bufs=6))
        craw = ctx.enter_context(tc.tile_pool(name="craw", bufs=1))

        def T(shape, dt, space=None, addr_space="Local", name=None):
            pool = dramp if space == "DRAM" else persist
            return pool.tile(shape, dt, tag=name, name=name, addr_space=addr_space)

        # ---- internal DRAM ----
        y_slice = [
            [T([RSIZE, F], BF16, space="DRAM", name=f"ysl{c}_{r}") for r in range(NREG)]
            for c in range(2)
        ]
        y_reg = [
            [
                T([TROWS, F], BF16, space="DRAM", addr_space="Shared",
                  name=f"yreg{c}_{r}")
                for r in range(NREG)
            ]
            for c in range(2)
        ]
        dram_sums = T([G + 8, F], F32, space="DRAM", name="dram_sums")
        ar_sums = T([G + 8, F], F32, space="DRAM", addr_space="Shared",
                    name="ar_sums")

        # ---- persistent SBUF ----
        hmid = T([P, NB * F], BF16, name="hmid")  # conv1 output
        y_nm = T([P, NB * F], BF16, name="y_nm")  # xw/(deg+1) for self-loop

        # constants
        iota_i = craw.tile([P, P], I32, tag="iota_i", name="iota_i")
        nc.gpsimd.iota(iota_i[:], pattern=[[1, P]], base=0, channel_multiplier=0)
        iota_f = T([P, P], F32, name="iota_f")
        nc.vector.tensor_copy(iota_f[:], iota_i[:])
        iota_bf = T([P, P], BF16, name="iota_bf")
        nc.vector.tensor_copy(iota_bf[:], iota_i[:])
        iota2_i = craw.tile([P, P], I32, tag="iota2_i", name="iota2_i")
        nc.gpsimd.iota(iota2_i[:], pattern=[[1, P]], base=128, channel_multiplier=0)
        iota2_bf = T([P, P], BF16, name="iota2_bf")
        nc.vector.tensor_copy(iota2_bf[:], iota2_i[:])
        id_f32 = T([P, P], F32, name="id_f32")
        make_identity(nc, id_f32[:])
        id_bf = T([P, P], BF16, name="id_bf")
        nc.vector.tensor_copy(id_bf[:], id_f32[:])
        ones_row = T([1, P], BF16, name="ones_row")
        nc.vector.memset(ones_row[:], 1.0)

        def load_cast(name, dram, shape, dt_in, dt_out):
            t = T(shape, dt_out, name=name)
            if dt_out == dt_in:
                nc.sync.dma_start(out=t[:], in_=dram[:])
            else:
                raw = craw.tile(shape, dt_in, tag=name + "_r", name=name + "_r")
                nc.sync.dma_start(out=raw[:], in_=dram[:])
                nc.vector.tensor_copy(t[:], raw[:])
            return t

        W1c = load_cast("W1", W1d, [F, F], F32, BF16)
        W2c = load_cast("W2", W2d, [F, F], F32, BF16)
        bc = [
            load_cast("b1", b1d, [1, F], F32, BF16),
            load_cast("b2", b2d, [1, F], F32, BF16),
        ]
        Wl_sb = load_cast("Wl", Wld, [F, NCLS], F32, BF16)
        bl_sb = load_cast("bl", bld, [1, NCLS], F32, BF16)
        bl0_sb = load_cast("bl0", bl0d, [P, NB], F32, F32)
        bl1_sb = load_cast("bl1", bl1d, [P, NB], F32, F32)
        cnt_sb = load_cast("cnt", cntd, [P, NGT], F32, F32)
        gix_sb = load_cast("gix", gixd, [P, 2], I32, I32)
        xv_sb = load_cast("xv", xvd, [P, NB * 4], BF16, BF16)

        deg_sb = craw.tile([P, NB], F32, tag="deg_sb", name="deg_sb")
        nc.sync.dma_start(out=deg_sb[:], in_=degd[:])
        rdeg = T([P, NB], F32, name="rdeg")  # 1/(deg+1)
        nc.vector.reciprocal(rdeg[:], deg_sb[:])

        # T'_j = tab_j @ W1 (zero-padded to [128,128] rows)
        tprime = []
        for j, (tab, rows) in enumerate(
            [(tabs[0], 16), (tabs[1], 16), (tabs[2], 128), (tabs[2], 128)]
        ):
            tp_sb = T([P, F], BF16, name=f"tprime{j}")
            if rows < P:
                nc.vector.memset(tp_sb[:], 0.0)
            traw = craw.tile([rows, F], F32, tag=f"tab{j}", name=f"tab{j}")
            r0 = (j - 2) * 128 if j >= 2 else 0
            nc.sync.dma_start(out=traw[:], in_=tab[r0 : r0 + rows, :])
            tbf = craw.tile([rows, F], BF16, tag=f"tabb{j}", name=f"tabb{j}")
            nc.vector.tensor_copy(tbf[:], traw[:])
            tps = tp_pool.tile([P, P], BF16, tag="tp")
            nc.tensor.transpose(tps[:, :rows], tbf[:], id_bf[:rows, :rows])
            tT = craw.tile([P, rows], BF16, tag=f"tabT{j}", name=f"tabT{j}")
            nc.scalar.copy(tT[:], tps[:, :rows])
            pps = xw_pool.tile([rows, F], F32, tag="xw")
            nc.tensor.matmul(pps[:], lhsT=tT[:, :], rhs=W1c[:], start=True, stop=True)
            nc.scalar.copy(tp_sb[:rows, :], pps[:])
            tprime.append(tp_sb)

        # zero dram_sums (pool scatter target) early
        zsb = craw.tile([P, 512], F32, tag="zsb", name="zsb")
        nc.vector.memset(zsb[:], 0.0)
        nrow = G + 8
        r_ = 0
        while r_ < nrow:
            take = min(512, ((nrow - r_) // P) * P)
            pp = P
            if take == 0:
                take = nrow - r_
                pp = take
            nc.sync.dma_start(
                out=dram_sums[r_ : r_ + take, :].rearrange("(c p) f -> p c f", p=pp),
                in_=zsb[:pp, : take * F // pp].rearrange("p (c f) -> p c f", f=F),
            )
            r_ += take

        # ---------------- conv phases ----------------
        def xw_tile_conv1(t, yreg_sb, yoff):
            """embed-fused xw1 for block t -> y_raw into yreg_sb, y_nm."""
            # natural-orientation one-hots oh[n, v] = (x_j[n] == v)
            ohn = sb_pool.tile([P, 4, P], BF16, tag="ohn")
            nc.vector.tensor_tensor(
                out=ohn[:, :3, :],
                in0=iota_bf[:].unsqueeze(1).broadcast_to([P, 3, P]),
                in1=xv_sb[:, t * 4 : t * 4 + 3].unsqueeze(2).broadcast_to([P, 3, P]),
                op=mybir.AluOpType.is_equal,
            )
            nc.vector.tensor_tensor(
                out=ohn[:, 3:4, :],
                in0=iota2_bf[:].unsqueeze(1).broadcast_to([P, 1, P]),
                in1=xv_sb[:, t * 4 + 3 : t * 4 + 4]
                .unsqueeze(2)
                .broadcast_to([P, 1, P]),
                op=mybir.AluOpType.is_equal,
            )
            # transpose all four one-hots into one PSUM tile, single copy out
            etp = etp_pool.tile([P, 4 * P], BF16, tag="etp")
            for j in range(4):
                nc.tensor.transpose(
                    etp[:, j * P : (j + 1) * P], ohn[:, j, :], id_bf[:]
                )
            ohT = sb_pool.tile([P, 4 * P], BF16, tag="ohT")
            nc.scalar.copy(ohT[:], etp[:])
            xwp = xw_pool.tile([P, F], F32, tag="xw")
            for j in range(4):
                nc.tensor.matmul(
                    xwp[:],
                    lhsT=ohT[:, j * P : (j + 1) * P],
                    rhs=tprime[j][:],
                    start=(j == 0),
                    stop=(j == 3),
                )
            nc.scalar.copy(yreg_sb[:, yoff * F : (yoff + 1) * F], xwp[:])
            nc.scalar.activation(
                y_nm[:, t * F : (t + 1) * F],
                xwp[:],
                mybir.ActivationFunctionType.Copy,
                scale=rdeg[:, t : t + 1],
            )

        def xw_tile_conv2(t, yreg_sb, yoff):
            tp = tp_pool.tile([P, P], BF16, tag="tp")
            nc.tensor.transpose(tp[:], hmid[:, t * F : (t + 1) * F], id_bf[:])
            hT = sb_pool.tile([P, P], BF16, tag="hT")
            nc.scalar.copy(hT[:], tp[:])
            xwp = xw_pool.tile([P, F], F32, tag="xw")
            nc.tensor.matmul(xwp[:], lhsT=hT[:], rhs=W2c[:], start=True, stop=True)
            nc.scalar.copy(yreg_sb[:, yoff * F : (yoff + 1) * F], xwp[:])
            nc.scalar.activation(
                y_nm[:, t * F : (t + 1) * F],
                xwp[:],
                mybir.ActivationFunctionType.Copy,
                scale=rdeg[:, t : t + 1],
            )

        pacc = [
            pacc_pool.tile([P, P], F32, tag=f"pacc{h}", name=f"pacc{h}")
            for h in range(2)
        ]
        bls = [bl0_sb, bl1_sb]

        for conv in range(2):
            # xw phase, region-sliced; AG_r fires as soon as region r ready
            for r in range(NREG):
                b0, b1_ = reg_blocks[r]
                nblk = b1_ - b0
                yreg_sb = yreg_pool.tile([P, 25 * F], BF16, tag="yreg")
                for t in range(b0, b1_):
                    if conv == 0:
                        xw_tile_conv1(t, yreg_sb, t - b0)
                    else:
                        xw_tile_conv2(t, yreg_sb, t - b0)
                nc.sync.dma_start(
                    out=y_slice[conv][r][: nblk * P, :].rearrange(
                        "(c p) f -> p c f", p=P
                    ),
                    in_=yreg_sb[:, : nblk * F].rearrange("p (c f) -> p c f", f=F),
                )
                if r == NREG - 1:
                    # bias row rides in region 3's unused tail (table row 2944)
                    nc.sync.dma_start(
                        out=y_slice[conv][r][23 * P : 23 * P + 1, :],
                        in_=bc[conv][:],
                    )
                nc.gpsimd.collective_compute(
                    "AllGather",
                    mybir.AluOpType.bypass,
                    replica_groups=RG,
                    ins=[y_slice[conv][r][:]],
                    outs=[y_reg[conv][r][:]],
                )

            # scatter phase
            col_off = 0
            for gi, blocks in enumerate(groups):
                nblk = len(blocks)
                nch_q = nblk * Cq
                nidx = nch_q * P
                msgs = []
                for r in range(NREG):
                    ixt = ix_pool.tile([P, GB * Cq * 8], I16, tag="ix")
                    nc.sync.dma_start(
                        out=ixt[:, : nidx // 16],
                        in_=xixd[:, col_off : col_off + nidx // 16],
                    )
                    col_off += nidx // 16
                    msg = msg_pool.tile([P, GB * Cq, F], BF16, tag="msg")
                    nc.gpsimd.dma_gather(
                        out_ap=msg[:, :nch_q, :],
                        in_ap=y_reg[conv][r][:, :],
                        idxs_ap=ixt[:, : nidx // 16],
                        num_idxs=nidx,
                        num_idxs_reg=nidx,
                        elem_size=F,
                        single_packet=False,
                        queue_num=r,
                    )
                    msgs.append(msg)
                for bi, b in enumerate(blocks):
                    ohs = oh_pool.tile([P, CPB * P], BF16, tag="oh")
                    nc.sync.dma_start(
                        out=ohs[:],
                        in_=ohd[:, b * CPB * P : (b + 1) * CPB * P],
                    )
                    acc = acc_pool.tile([P, P], F32, tag="acc")
                    nc.tensor.matmul(
                        acc[:],
                        lhsT=id_bf[:],
                        rhs=y_nm[:, b * F : (b + 1) * F],
                        start=True,
                        stop=False,
                    )
                    for r in range(NREG):
                        for cc in range(Cq):
                            j = r * Cq + cc
                            nc.tensor.matmul(
                                acc[:],
                                lhsT=ohs[:, j * P : (j + 1) * P],
                                rhs=msgs[r][:, bi * Cq + cc, :],
                                start=False,
                                stop=(j == CPB - 1),
                            )
                    if conv == 0:
                        nc.scalar.activation(
                            hmid[:, b * F : (b + 1) * F],
                            acc[:],
                            mybir.ActivationFunctionType.Relu,
                        )
                    else:
                        htmp = sb_pool.tile([P, P], BF16, tag="htmp")
                        nc.scalar.activation(
                            htmp[:], acc[:], mybir.ActivationFunctionType.Relu
                        )
                        for h in range(2):
                            oht = sb_pool.tile([P, P], BF16, tag="pooloh")
                            nc.vector.tensor_tensor(
                                out=oht[:],
                                in0=iota_f[:],
                                in1=bls[h][:, b : b + 1].to_broadcast([P, P]),
                                op=mybir.AluOpType.is_equal,
                            )
                            nc.tensor.matmul(
                                pacc[h][:],
                                lhsT=oht[:],
                                rhs=htmp[:],
                                start=(b == 0),
                                stop=(b == NB - 1),
                            )

        # ---------------- global mean pool + linear ----------------
        for h in range(2):
            se = sb_pool.tile([P, P], F32, tag="ep")
            nc.vector.tensor_copy(se[:], pacc[h][:])
            nc.gpsimd.indirect_dma_start(
                out=dram_sums[:],
                out_offset=bass.IndirectOffsetOnAxis(ap=gix_sb[:, h : h + 1], axis=0),
                in_=se[:],
                in_offset=None,
            )
        nc.gpsimd.collective_compute(
            "AllReduce",
            mybir.AluOpType.add,
            replica_groups=RG,
            ins=[dram_sums[:]],
            outs=[ar_sums[:]],
        )
        recip = T([P, NGT], F32, name="recip")
        nc.vector.reciprocal(recip[:], cnt_sb[:])
        for t in range(NGT):
            art = sb_pool.tile([P, P], F32, tag="art")
            nc.sync.dma_start(out=art[:], in_=ar_sums[t * P : (t + 1) * P, :])
            pooled = sb_pool.tile([P, P], BF16, tag="ep")
            nc.vector.tensor_tensor(
                out=pooled[:],
                in0=art[:],
                in1=recip[:, t : t + 1].to_broadcast([P, P]),
                op=mybir.AluOpType.mult,
            )
            ptp = tp_pool.tile([P, P], BF16, tag="tp")
            nc.tensor.transpose(ptp[:], pooled[:], id_bf[:])
            pooledT = sb_pool.tile([P, P], BF16, tag="hT2")
            nc.scalar.copy(pooledT[:], ptp[:])
            op = xw_pool.tile([P, NCLS], F32, tag="xw")
            nc.tensor.matmul(op[:], lhsT=pooledT[:], rhs=Wl_sb[:], start=True, stop=False)
            nc.tensor.matmul(op[:], lhsT=ones_row[:], rhs=bl_sb[:], start=False, stop=True)
            oute = sb_pool.tile([P, NCLS], F32, tag="oute")
            nc.vector.tensor_copy(oute[:], op[:])
            nc.sync.dma_start(out=outd[t * P : (t + 1) * P, :], in_=oute[:])

        ctx.close()
    nc.compile()
    return nc


_CACHE = {}


def _get_nc(cfg, Cq, pl, totcol, nohc):
    key = (tuple(sorted(cfg.items())), Cq, totcol, nohc)
    if key not in _CACHE:
        _CACHE[key] = _build(cfg, Cq, pl, totcol, nohc)
    return _CACHE[key]


def run(inputs, cfg, trace=False):
    x = np.asarray(inputs["x"])
    per_core, Cq, pl = _prep_host(
        x, np.asarray(inputs["edge_index"]), np.asarray(inputs["batch"]), cfg
    )
    totcol = per_core[0]["edge_idx"].shape[1]
    nohc = per_core[0]["oh"].shape[1]
    nc = _get_nc(cfg, Cq, pl, totcol, nohc)

    shared = dict(
        shape_tab=np.asarray(inputs["shape_tab"], np.float32),
        color_tab=np.asarray(inputs["color_tab"], np.float32),
        pos_tab=np.asarray(inputs["pos_tab"], np.float32),
        W1=np.asarray(inputs["W1"], np.float32),
        W2=np.asarray(inputs["W2"], np.float32),
        b1=np.asarray(inputs["b1"], np.float32).reshape(1, -1),
        b2=np.asarray(inputs["b2"], np.float32).reshape(1, -1),
        Wlin=np.asarray(inputs["Wlin"], np.float32),
        blin=np.asarray(inputs["blin"], np.float32).reshape(1, -1),
    )
    in_maps = [{**shared, **per_core[k]} for k in range(NCORES)]
    res = run_bass_kernel_spmd(nc, in_maps, list(range(NCORES)), trace=trace)
    out = np.asarray(res.results[0]["out"], np.float32)
    return out, res


def kernel(**inputs) -> np.ndarray:
    out, _ = run(inputs, CFG)
    return out


# revision 36
# speedup vs baseline: 1.0971x; 1.0451x over previous
"""GCN classifier (2x GCNConv + mean-pool + linear) on 8 Trainium2 NeuronCores.

v2: gather-descriptor-roofline design.
  - nodes (and incident edges, partitioned by dst) sharded over 8 cores;
    128x128 weights replicated; SPMD single NEFF.
  - per-edge messages fetched with dma_gather from a per-region (quarter
    of the node space, <32768 rows for int16 idx) AllGather'd feature
    table; 4 SWDGE queues (one per region) kept saturated.
  - scatter to dst via one-hot matmuls whose one-hot matrices are built
    on HOST (values = dinv[src]*dinv[dst], bias slot = row of ones that
    multiplies a `b` row appended to the table) and streamed from DRAM
    with plain HWDGE DMA. Self-loop added with an identity matmul; the
    whole conv epilogue is one Relu activation.
  - embedding is folded into conv1's xw: xw1 = sum_j OH_j @ (tab_j@W1),
    with OH_j built on-device from x values (no dma_gather).
  - AllGather sliced into 4 region collectives, pipelined with compute.
  - mean-pool accumulation fused into conv2's epilogue.
"""

import math
import sys

sys.path.insert(0, "/opt/trn_rl_repo")

import ml_dtypes
import numpy as np

import concourse.bass as bass
import concourse.mybir as mybir
import concourse.tile as tile
from concourse import bacc
from concourse.bass_utils import run_bass_kernel_spmd
from concourse.masks import make_identity

BF16 = mybir.dt.bfloat16
F32 = mybir.dt.float32
I16 = mybir.dt.int16
I32 = mybir.dt.int32
NP_BF16 = ml_dtypes.bfloat16

P = 128
NCORES = 8

CFG = dict(N=100_000, E=1_600_000, G=1024, F=128, NCLS=10)

GB = 6  # dst blocks per gather group
RSIZE = 3200  # region rows per core slice (25 blocks; last region 23 blocks)
NREG = 4


def _plan(cfg):
    N, G = cfg["N"], cfg["G"]
    p = {}
    p["NPC"] = N // NCORES  # 12500 nodes per core
    p["NB"] = math.ceil(p["NPC"] / P)  # 98 blocks
    p["NBP"] = p["NB"] * P  # 12544
    # region r covers blocks [25r, 25r+25) (last: 23); rows [3200r, ...)
    p["reg_blocks"] = [(25 * r, min(25 * r + 25, p["NB"])) for r in range(NREG)]
    p["TROWS"] = NCORES * RSIZE  # 25600 rows per region table
    assert p["TROWS"] < 32768
    # region 3 only has 2944 data rows per core; row 2944 of core 0's slice
    # (= table row 2944 of region 3) carries the conv bias vector.
    p["BIAS_ROW"] = 23 * P
    p["groups"] = [
        list(range(g, min(g + GB, p["NB"]))) for g in range(0, p["NB"], GB)
    ]
    p["NGT"] = G // P
    assert G % P == 0
    return p


def _wrap16(idx_flat):
    """int16 index list -> [128, n/16] wrapped in 16 partitions, replicated 8x."""
    return np.tile(idx_flat.reshape(-1, 16).T, (8, 1))


def _prep_host(x, edge_index, batch, cfg):
    """Integer/index preprocessing + per-core metadata. Returns (per_core, Cq, pl)."""
    pl = _plan(cfg)
    N, G = cfg["N"], cfg["G"]
    NPC, NB, NBP = pl["NPC"], pl["NB"], pl["NBP"]

    src = np.asarray(edge_index[0], np.int64)
    dst = np.asarray(edge_index[1], np.int64)
    batch = np.asarray(batch, np.int64)
    x = np.asarray(x, np.int64)

    deg_p1 = (np.bincount(dst, minlength=N) + 1).astype(np.float64)
    dinv = 1.0 / np.sqrt(deg_p1)  # global normalization coefs
    cnt = np.maximum(np.bincount(batch, minlength=G), 1).astype(np.float32)
    cnt_pt = cnt.reshape(pl["NGT"], P).T.copy()  # [P, NGT]

    # region-local row index of a global src node
    s_core = src // NPC
    s_off = src % NPC
    s_reg = np.minimum(s_off // RSIZE, NREG - 1)
    s_row = s_core * RSIZE + (s_off - s_reg * RSIZE)  # < 25600

    core_of = dst // NPC
    per_core_edge = []
    Cq = 1
    for k in range(NCORES):
        m = core_of == k
        d_k = dst[m] - k * NPC
        blk = d_k >> 7
        key = (blk * NREG + s_reg[m]).astype(np.int64)
        order = np.argsort(key, kind="stable")
        counts = np.bincount(key, minlength=NB * NREG)
        Cq = max(Cq, math.ceil(counts.max() / P))
        sloc = s_row[m].astype(np.int16)[order]
        dslot = (d_k & 127).astype(np.int16)[order]
        coef = (dinv[src[m]] * dinv[dst[m]]).astype(np.float32)[order]
        per_core_edge.append((sloc, dslot, coef, counts))

    cap = Cq * P
    per_core = []
    for k in range(NCORES):
        sloc, dslot, coef, counts = per_core_edge[k]
        src_pad = np.zeros((NB * NREG, cap), np.int16)
        dst_pad = np.full((NB * NREG, cap), -1, np.int16)
        coef_pad = np.zeros((NB * NREG, cap), np.float32)
        starts = np.concatenate([[0], np.cumsum(counts)])
        for cell in range(NB * NREG):
            c0, c1 = starts[cell], starts[cell + 1]
            n = c1 - c0
            if n:
                src_pad[cell, :n] = sloc[c0:c1]
                dst_pad[cell, :n] = dslot[c0:c1]
                coef_pad[cell, :n] = coef[c0:c1]

        # bias slot: one pad slot per block (in its region-3 cell, whose table
        # row BIAS_ROW holds b) -> gathers the b row; one-hot row = all ones
        # so every dst of the block receives +b[f].
        BIAS = pl["BIAS_ROW"]
        for b in range(NB):
            cell = b * NREG + (NREG - 1)
            n = counts[cell]
            assert n < cap, f"core {k} block {b}: no pad slot for bias"
            src_pad[cell, n] = BIAS
            dst_pad[cell, n] = -2  # marker: full row of ones

        # gather index stream per (group, region): concat cell streams
        idx_cols = []
        for blocks in pl["groups"]:
            for r in range(NREG):
                cells = [b * NREG + r for b in blocks]
                flat = src_pad[cells].reshape(-1)
                idx_cols.append(_wrap16(flat))
        edge_idx = np.concatenate(idx_cols, 1)  # [128, totcol] i16

        # host-built one-hot matrices, block-major, chunk order (r, cc):
        # oh[p, b*4Cq*128 + (r*Cq+cc)*128 + i] = coef of edge slot p of that
        # chunk if its dst slot == i (or 1.0 for the bias slot's full row).
        dst3 = dst_pad.reshape(NB, NREG * Cq, P)  # [b, chunk, p]
        coef3 = coef_pad.reshape(NB, NREG * Cq, P)
        iota = np.arange(P, dtype=np.int16)
        onehot = (dst3[:, :, :, None] == iota[None, None, None, :]).astype(
            np.float32
        )
        onehot *= coef3[:, :, :, None]
        onehot += (dst3[:, :, :, None] == -2).astype(np.float32)
        # -> [p, b, chunk, i] -> [128, NB*4Cq*128]
        oh = (
            onehot.transpose(2, 0, 1, 3)
            .reshape(P, NB * NREG * Cq * P)
            .astype(NP_BF16)
        )

        # degree (layout [p, c] = local node c*128+p), pad nodes -> deg+1 = 1
        dp = np.ones(NBP, np.float32)
        dp[: NPC] = deg_p1[k * NPC : (k + 1) * NPC].astype(np.float32)
        dp = dp.reshape(NB, P).T.copy()

        # pool metadata
        bl = batch[k * NPC : (k + 1) * NPC]
        gbase = int(bl[0])
        gspan = int(bl[-1]) - gbase + 1
        assert gspan <= 2 * P, f"core {k} graph span {gspan} > 256"
        blf = np.full(NBP, -1.0, np.float32)
        blf[: NPC] = (bl - gbase).astype(np.float32)
        bl0 = blf.reshape(NB, P).T.astype(np.float32)
        bl1 = (blf - P).reshape(NB, P).T.astype(np.float32)
        gidx = np.zeros((P, 2), np.int32)
        for h in range(2):
            v = gbase + h * P + np.arange(P)
            v = np.where(v < G, v, G + (v % 8))
            gidx[:, h] = v

        # embedding gather indices into the combined [288, F] bf16 table:
        # per region, j-major (shape rows | color rows+16 | pos rows+32)
        xi = np.zeros((NBP, 3), np.int16)
        xi[: NPC] = x[k * NPC : (k + 1) * NPC].astype(np.int16)
        xi[:, 1] += 16
        xi[:, 2] += 32
        emb_cols = []
        for b0, b1_ in pl["reg_blocks"]:
            nt = b1_ - b0
            seg = np.zeros((3, 25 * P), np.int16)
            seg[:, : nt * P] = xi[b0 * P : b1_ * P].T
            emb_cols.append(_wrap16(seg.reshape(-1)))
        emb_idx = np.concatenate(emb_cols, 1)  # [128, 4*75*128/16]

        per_core.append(
            dict(
                deg_p1=dp,
                bl0=bl0,
                bl1=bl1,
                gidx=gidx,
                cnt=cnt_pt,
                emb_idx=emb_idx,
                edge_idx=edge_idx,
                oh=oh,
            )
        )
    return per_core, Cq, pl


def _build(cfg, Cq, pl, totcol, nohc):
    """Build the SPMD Bass program (one NEFF for all 8 cores)."""
    N, G, F, NCLS = cfg["N"], cfg["G"], cfg["F"], cfg["NCLS"]
    NPC, NB, NGT = pl["NPC"], pl["NB"], pl["NGT"]
    groups, reg_blocks, TROWS = pl["groups"], pl["reg_blocks"], pl["TROWS"]
    CPB = NREG * Cq  # chunks per block

    nc = bacc.Bacc("TRN2", num_devices=NCORES, num_swdge_queues=4)
    RG = [list(range(NCORES))]

    # ---- I/O ----
    tabs = [
        nc.dram_tensor("shape_tab", [16, F], F32, kind="ExternalInput"),
        nc.dram_tensor("color_tab", [16, F], F32, kind="ExternalInput"),
        nc.dram_tensor("pos_tab", [256, F], F32, kind="ExternalInput"),
    ]
    W1d = nc.dram_tensor("W1", [F, F], F32, kind="ExternalInput")
    W2d = nc.dram_tensor("W2", [F, F], F32, kind="ExternalInput")
    b1d = nc.dram_tensor("b1", [1, F], F32, kind="ExternalInput")
    b2d = nc.dram_tensor("b2", [1, F], F32, kind="ExternalInput")
    Wld = nc.dram_tensor("Wlin", [F, NCLS], F32, kind="ExternalInput")
    bld = nc.dram_tensor("blin", [1, NCLS], F32, kind="ExternalInput")
    degd = nc.dram_tensor("deg_p1", [P, NB], F32, kind="ExternalInput")
    bl0d = nc.dram_tensor("bl0", [P, NB], F32, kind="ExternalInput")
    bl1d = nc.dram_tensor("bl1", [P, NB], F32, kind="ExternalInput")
    gixd = nc.dram_tensor("gidx", [P, 2], I32, kind="ExternalInput")
    cntd = nc.dram_tensor("cnt", [P, NGT], F32, kind="ExternalInput")
    eixd = nc.dram_tensor("emb_idx", [P, NREG * 3 * 25 * 8], I16, kind="ExternalInput")
    xixd = nc.dram_tensor("edge_idx", [P, totcol], I16, kind="ExternalInput")
    ohd = nc.dram_tensor("oh", [P, nohc], BF16, kind="ExternalInput")
    outd = nc.dram_tensor("out", [G, NCLS], F32, kind="ExternalOutput")

    with tile.TileContext(nc) as tc:
        import contextlib

        ctx = contextlib.ExitStack()
        persist = ctx.enter_context(tc.tile_pool(name="persist", bufs=1))
        dramp = ctx.enter_context(tc.tile_pool(name="dramp", bufs=1, space="DRAM"))
        xw_pool = ctx.enter_context(tc.tile_pool(name="xw", bufs=2, space="PSUM"))
        tp_pool = ctx.enter_context(tc.tile_pool(name="tp", bufs=1, space="PSUM"))
        acc_pool = ctx.enter_context(tc.tile_pool(name="acc", bufs=3, space="PSUM"))
        pacc_pool = ctx.enter_context(tc.tile_pool(name="pacc", bufs=1, space="PSUM"))
        sb_pool = ctx.enter_context(tc.tile_pool(name="work", bufs=4))
        hreg_pool = ctx.enter_context(tc.tile_pool(name="hreg", bufs=1))
        yreg_pool = ctx.enter_context(tc.tile_pool(name="yreg", bufs=2))
        msg_pool = ctx.enter_context(tc.tile_pool(name="msg", bufs=9))
        oh_pool = ctx.enter_context(tc.tile_pool(name="oh", bufs=3))
        ix_pool = ctx.enter_context(tc.tile_pool(name="ix", bufs=6))
        craw = ctx.enter_context(tc.tile_pool(name="craw", bufs=1))

        def T(shape, dt, space=None, addr_space="Local", name=None):
            pool = dramp if space == "DRAM" else persist
            return pool.tile(shape, dt, tag=name, name=name, addr_space=addr_space)

        # ---- internal DRAM ----
        y_slice = [
            [T([RSIZE, F], BF16, space="DRAM", name=f"ysl{c}_{r}") for r in range(NREG)]
            for c in range(2)
        ]
        y_reg = [
            [
                T([TROWS, F], BF16, space="DRAM", addr_space="Shared",
                  name=f"yreg{c}_{r}")
                for r in range(NREG)
            ]
            for c in range(2)
        ]
        dram_sums = T([G + 8, F], F32, space="DRAM", name="dram_sums")
        ar_sums = T([G + 8, F], F32, space="DRAM", addr_space="Shared",
                    name="ar_sums")

        # ---- persistent SBUF ----
        hmid = T([P, NB * F], BF16, name="hmid")  # conv1 output
        y_nm = T([P, NB * F], BF16, name="y_nm")  # xw/(deg+1) for self-loop

        # constants
        iota_i = craw.tile([P, P], I32, tag="iota_i", name="iota_i")
        nc.gpsimd.iota(iota_i[:], pattern=[[1, P]], base=0, channel_multiplier=0)
        iota_f = T([P, P], F32, name="iota_f")
        nc.vector.tensor_copy(iota_f[:], iota_i[:])
        id_f32 = T([P, P], F32, name="id_f32")
        make_identity(nc, id_f32[:])
        id_bf = T([P, P], BF16, name="id_bf")
        nc.vector.tensor_copy(id_bf[:], id_f32[:])
        ones_row = T([1, P], BF16, name="ones_row")
        nc.vector.memset(ones_row[:], 1.0)

        def load_cast(name, dram, shape, dt_in, dt_out):
            t = T(shape, dt_out, name=name)
            if dt_out == dt_in:
                nc.sync.dma_start(out=t[:], in_=dram[:])
            else:
                raw = craw.tile(shape, dt_in, tag=name + "_r", name=name + "_r")
                nc.sync.dma_start(out=raw[:], in_=dram[:])
                nc.vector.tensor_copy(t[:], raw[:])
            return t

        W1c = load_cast("W1", W1d, [F, F], F32, BF16)
        W2c = load_cast("W2", W2d, [F, F], F32, BF16)
        bc = [
            load_cast("b1", b1d, [1, F], F32, BF16),
            load_cast("b2", b2d, [1, F], F32, BF16),
        ]
        Wl_sb = load_cast("Wl", Wld, [F, NCLS], F32, BF16)
        bl_sb = load_cast("bl", bld, [1, NCLS], F32, BF16)
        bl0_sb = load_cast("bl0", bl0d, [P, NB], F32, F32)
        bl1_sb = load_cast("bl1", bl1d, [P, NB], F32, F32)
        cnt_sb = load_cast("cnt", cntd, [P, NGT], F32, F32)
        gix_sb = load_cast("gix", gixd, [P, 2], I32, I32)
        eix_sb = load_cast("eix", eixd, [P, NREG * 3 * 25 * 8], I16, I16)

        deg_sb = craw.tile([P, NB], F32, tag="deg_sb", name="deg_sb")
        nc.sync.dma_start(out=deg_sb[:], in_=degd[:])
        rdeg = T([P, NB], F32, name="rdeg")  # 1/(deg+1)
        nc.vector.reciprocal(rdeg[:], deg_sb[:])

        # combined bf16 embedding table in DRAM: rows 0-15 shape, 16-31 color,
        # 32-287 pos
        tab_bf = T([288, F], BF16, space="DRAM", name="tab_bf")
        for j, (tab, rows, r0) in enumerate(
            [(tabs[0], 16, 0), (tabs[1], 16, 16), (tabs[2], 128, 32), (tabs[2], 128, 160)]
        ):
            traw = craw.tile([rows, F], F32, tag=f"tab{j}", name=f"tab{j}")
            nc.sync.dma_start(out=traw[:], in_=tab[r0 - 32 : r0 - 32 + rows, :] if j >= 2 else tab[:rows, :])
            tbf = craw.tile([rows, F], BF16, tag=f"tabb{j}", name=f"tabb{j}")
            nc.vector.tensor_copy(tbf[:], traw[:])
            nc.sync.dma_start(out=tab_bf[r0 : r0 + rows, :], in_=tbf[:])

        # zero dram_sums (pool scatter target) early
        zsb = craw.tile([P, 512], F32, tag="zsb", name="zsb")
        nc.vector.memset(zsb[:], 0.0)
        nrow = G + 8
        r_ = 0
        while r_ < nrow:
            take = min(512, ((nrow - r_) // P) * P)
            pp = P
            if take == 0:
                take = nrow - r_
                pp = take
            nc.sync.dma_start(
                out=dram_sums[r_ : r_ + take, :].rearrange("(c p) f -> p c f", p=pp),
                in_=zsb[:pp, : take * F // pp].rearrange("p (c f) -> p c f", f=F),
            )
            r_ += take

        # ---------------- conv phases ----------------
        def xw_tile(t, hsrc, Wc_, yreg_sb, yoff):
            """y_raw = h @ W for block t -> yreg_sb col yoff; y_nm = y_raw/(deg+1)."""
            tp = tp_pool.tile([P, P], BF16, tag="tp")
            nc.tensor.transpose(tp[:], hsrc, id_bf[:])
            hT = sb_pool.tile([P, P], BF16, tag="hT")
            nc.scalar.copy(hT[:], tp[:])
            xwp = xw_pool.tile([P, F], F32, tag="xw")
            nc.tensor.matmul(xwp[:], lhsT=hT[:], rhs=Wc_[:], start=True, stop=True)
            nc.scalar.copy(yreg_sb[:, yoff * F : (yoff + 1) * F], xwp[:])
            nc.scalar.activation(
                y_nm[:, t * F : (t + 1) * F],
                xwp[:],
                mybir.ActivationFunctionType.Copy,
                scale=rdeg[:, t : t + 1],
            )

        pacc = [
            pacc_pool.tile([P, P], F32, tag=f"pacc{h}", name=f"pacc{h}")
            for h in range(2)
        ]
        bls = [bl0_sb, bl1_sb]

        for conv in range(2):
            # xw phase, region-sliced; AG_r fires as soon as region r ready
            for r in range(NREG):
                b0, b1_ = reg_blocks[r]
                nblk = b1_ - b0
                if conv == 0:
                    # embedding: gather 3 rows/node from the combined table,
                    # sum into the region's h tile (frees the gather buffer
                    # for the next region's gather to overlap with xw)
                    ne = 3 * 25 * P
                    hraw = hreg_pool.tile([P, 3, 25, F], BF16, tag="hraw")
                    nc.gpsimd.dma_gather(
                        out_ap=hraw[:].rearrange("p a b f -> p (a b) f"),
                        in_ap=tab_bf[:, :],
                        idxs_ap=eix_sb[:, r * ne // 16 : (r + 1) * ne // 16],
                        num_idxs=ne,
                        num_idxs_reg=ne,
                        elem_size=F,
                        single_packet=False,
                        queue_num=r,
                    )
                    h25 = yreg_pool.tile([P, 25, F], BF16, tag="h25")
                    nc.vector.tensor_tensor(
                        out=h25[:], in0=hraw[:, 0], in1=hraw[:, 1],
                        op=mybir.AluOpType.add,
                    )
                    nc.vector.tensor_tensor(
                        out=h25[:], in0=h25[:], in1=hraw[:, 2],
                        op=mybir.AluOpType.add,
                    )
                yreg_sb = yreg_pool.tile([P, 25 * F], BF16, tag="yreg")
                for t in range(b0, b1_):
                    if conv == 0:
                        hsrc = h25[:, t - b0, :]
                        xw_tile(t, hsrc, W1c, yreg_sb, t - b0)
                    else:
                        hsrc = hmid[:, t * F : (t + 1) * F]
                        xw_tile(t, hsrc, W2c, yreg_sb, t - b0)
                nc.sync.dma_start(
                    out=y_slice[conv][r][: nblk * P, :].rearrange(
                        "(c p) f -> p c f", p=P
                    ),
                    in_=yreg_sb[:, : nblk * F].rearrange("p (c f) -> p c f", f=F),
                )
                if r == NREG - 1:
                    # bias row rides in region 3's unused tail (table row 2944)
                    nc.sync.dma_start(
                        out=y_slice[conv][r][23 * P : 23 * P + 1, :],
                        in_=bc[conv][:],
                    )
                nc.gpsimd.collective_compute(
                    "AllGather",
                    mybir.AluOpType.bypass,
                    replica_groups=RG,
                    ins=[y_slice[conv][r][:]],
                    outs=[y_reg[conv][r][:]],
                )

            # scatter phase
            col_off = 0
            for gi, blocks in enumerate(groups):
                nblk = len(blocks)
                nch_q = nblk * Cq
                nidx = nch_q * P
                msgs = []
                for r in range(NREG):
                    ixt = ix_pool.tile([P, GB * Cq * 8], I16, tag="ix")
                    nc.sync.dma_start(
                        out=ixt[:, : nidx // 16],
                        in_=xixd[:, col_off : col_off + nidx // 16],
                    )
                    col_off += nidx // 16
                    msg = msg_pool.tile([P, GB * Cq, F], BF16, tag="msg")
                    nc.gpsimd.dma_gather(
                        out_ap=msg[:, :nch_q, :],
                        in_ap=y_reg[conv][r][:, :],
                        idxs_ap=ixt[:, : nidx // 16],
                        num_idxs=nidx,
                        num_idxs_reg=nidx,
                        elem_size=F,
                        single_packet=False,
                        queue_num=r,
                    )
                    msgs.append(msg)
                for bi, b in enumerate(blocks):
                    # oh loads ride the scalar engine's HWDGE ring so they
                    # don't serialize behind idx/y DMAs on the sync ring
                    ohs = oh_pool.tile([P, CPB * P], BF16, tag="oh")
                    nc.scalar.dma_start(
                        out=ohs[:],
                        in_=ohd[:, b * CPB * P : (b + 1) * CPB * P],
                    )
                    acc = acc_pool.tile([P, P], F32, tag="acc")
                    nc.tensor.matmul(
                        acc[:],
                        lhsT=id_bf[:],
                        rhs=y_nm[:, b * F : (b + 1) * F],
                        start=True,
                        stop=False,
                    )
                    for r in range(NREG):
                        for cc in range(Cq):
                            j = r * Cq + cc
                            nc.tensor.matmul(
                                acc[:],
                                lhsT=ohs[:, j * P : (j + 1) * P],
                                rhs=msgs[r][:, bi * Cq + cc, :],
                                start=False,
                                stop=(j == CPB - 1),
                            )
                    if conv == 0:
                        nc.scalar.activation(
                            hmid[:, b * F : (b + 1) * F],
                            acc[:],
                            mybir.ActivationFunctionType.Relu,
                        )
                    else:
                        htmp = sb_pool.tile([P, P], BF16, tag="htmp")
                        nc.scalar.activation(
                            htmp[:], acc[:], mybir.ActivationFunctionType.Relu
                        )
                        for h in range(2):
                            oht = sb_pool.tile([P, P], BF16, tag="pooloh")
                            nc.vector.tensor_tensor(
                                out=oht[:],
                                in0=iota_f[:],
                                in1=bls[h][:, b : b + 1].to_broadcast([P, P]),
                                op=mybir.AluOpType.is_equal,
                            )
                            nc.tensor.matmul(
                                pacc[h][:],
                                lhsT=oht[:],
                                rhs=htmp[:],
                                start=(b == 0),
                                stop=(b == NB - 1),
                            )

        # ---------------- global mean pool + linear ----------------
        for h in range(2):
            se = sb_pool.tile([P, P], F32, tag="ep")
            nc.vector.tensor_copy(se[:], pacc[h][:])
            nc.gpsimd.indirect_dma_start(
                out=dram_sums[:],
                out_offset=bass.IndirectOffsetOnAxis(ap=gix_sb[:, h : h + 1], axis=0),
                in_=se[:],
                in_offset=None,
            )
        nc.gpsimd.collective_compute(
            "AllReduce",
            mybir.AluOpType.add,
            replica_groups=RG,
            ins=[dram_sums[:]],
            outs=[ar_sums[:]],
        )
        recip = T([P, NGT], F32, name="recip")
        nc.vector.reciprocal(recip[:], cnt_sb[:])
        for t in range(NGT):
            art = sb_pool.tile([P, P], F32, tag="art")
            nc.sync.dma_start(out=art[:], in_=ar_sums[t * P : (t + 1) * P, :])
            pooled = sb_pool.tile([P, P], BF16, tag="ep")
            nc.vector.tensor_tensor(
                out=pooled[:],
                in0=art[:],
                in1=recip[:, t : t + 1].to_broadcast([P, P]),
                op=mybir.AluOpType.mult,
            )
            ptp = tp_pool.tile([P, P], BF16, tag="tp")
            nc.tensor.transpose(ptp[:], pooled[:], id_bf[:])
            pooledT = sb_pool.tile([P, P], BF16, tag="hT2")
            nc.scalar.copy(pooledT[:], ptp[:])
            op = xw_pool.tile([P, NCLS], F32, tag="xw")
            nc.tensor.matmul(op[:], lhsT=pooledT[:], rhs=Wl_sb[:], start=True, stop=False)
            nc.tensor.matmul(op[:], lhsT=ones_row[:], rhs=bl_sb[:], start=False, stop=True)
            oute = sb_pool.tile([P, NCLS], F32, tag="oute")
            nc.vector.tensor_copy(oute[:], op[:])
            nc.sync.dma_start(out=outd[t * P : (t + 1) * P, :], in_=oute[:])

        ctx.close()
    nc.compile()
    return nc


_CACHE = {}


def _get_nc(cfg, Cq, pl, totcol, nohc):
    key = (tuple(sorted(cfg.items())), Cq, totcol, nohc)
    if key not in _CACHE:
        _CACHE[key] = _build(cfg, Cq, pl, totcol, nohc)
    return _CACHE[key]


def run(inputs, cfg, trace=False):
    x = np.asarray(inputs["x"])
    per_core, Cq, pl = _prep_host(
        x, np.asarray(inputs["edge_index"]), np.asarray(inputs["batch"]), cfg
    )
    totcol = per_core[0]["edge_idx"].shape[1]
    nohc = per_core[0]["oh"].shape[1]
    nc = _get_nc(cfg, Cq, pl, totcol, nohc)

    shared = dict(
        shape_tab=np.asarray(inputs["shape_tab"], np.float32),
        color_tab=np.asarray(inputs["color_tab"], np.float32),
        pos_tab=np.asarray(inputs["pos_tab"], np.float32),
        W1=np.asarray(inputs["W1"], np.float32),
        W2=np.asarray(inputs["W2"], np.float32),
        b1=np.asarray(inputs["b1"], np.float32).reshape(1, -1),
        b2=np.asarray(inputs["b2"], np.float32).reshape(1, -1),
        Wlin=np.asarray(inputs["Wlin"], np.float32),
        blin=np.asarray(inputs["blin"], np.float32).reshape(1, -1),
    )
    in_maps = [{**shared, **per_core[k]} for k in range(NCORES)]
    res = run_bass_kernel_spmd(nc, in_maps, list(range(NCORES)), trace=trace)
    out = np.asarray(res.results[0]["out"], np.float32)
    return out, res


def kernel(**inputs) -> np.ndarray:
    out, _ = run(inputs, CFG)
    return out


# revision 46
# speedup vs baseline: 1.2592x; 1.1477x over previous
"""GCN classifier (2x GCNConv + mean-pool + linear) on 8 Trainium2 NeuronCores.

v2: gather-descriptor-roofline design.
  - nodes (and incident edges, partitioned by dst) sharded over 8 cores;
    128x128 weights replicated; SPMD single NEFF.
  - per-edge messages fetched with dma_gather from a per-region (quarter
    of the node space, <32768 rows for int16 idx) AllGather'd feature
    table; 4 SWDGE queues (one per region) kept saturated.
  - scatter to dst via one-hot matmuls whose one-hot matrices are built
    on HOST (values = dinv[src]*dinv[dst], bias slot = row of ones that
    multiplies a `b` row appended to the table) and streamed from DRAM
    with plain HWDGE DMA. Self-loop added with an identity matmul; the
    whole conv epilogue is one Relu activation.
  - embedding is folded into conv1's xw: xw1 = sum_j OH_j @ (tab_j@W1),
    with OH_j built on-device from x values (no dma_gather).
  - AllGather sliced into 4 region collectives, pipelined with compute.
  - mean-pool accumulation fused into conv2's epilogue.
"""

import math
import sys

sys.path.insert(0, "/opt/trn_rl_repo")

import ml_dtypes
import numpy as np

import concourse.bass as bass
import concourse.mybir as mybir
import concourse.tile as tile
from concourse import bacc
from concourse.bass_utils import run_bass_kernel_spmd
from concourse.masks import make_identity

BF16 = mybir.dt.bfloat16
FP8 = mybir.dt.float8e4
F32 = mybir.dt.float32
I16 = mybir.dt.int16
I32 = mybir.dt.int32
NP_BF16 = ml_dtypes.bfloat16
NP_FP8 = ml_dtypes.float8_e4m3

P = 128
NCORES = 8

CFG = dict(N=100_000, E=1_600_000, G=1024, F=128, NCLS=10)

GB = 6  # dst blocks per gather group
RSIZE = 3200  # region rows per core slice (25 blocks; last region 23 blocks)
NREG = 4


def _plan(cfg):
    N, G = cfg["N"], cfg["G"]
    p = {}
    p["NPC"] = N // NCORES  # 12500 nodes per core
    p["NB"] = math.ceil(p["NPC"] / P)  # 98 blocks
    p["NBP"] = p["NB"] * P  # 12544
    # region r covers blocks [25r, 25r+25) (last: 23); rows [3200r, ...)
    p["reg_blocks"] = [(25 * r, min(25 * r + 25, p["NB"])) for r in range(NREG)]
    p["TROWS"] = NCORES * RSIZE  # 25600 rows per region table
    assert p["TROWS"] < 32768
    # region 3 only has 2944 data rows per core; row 2944 of core 0's slice
    # (= table row 2944 of region 3) carries the conv bias vector.
    p["BIAS_ROW"] = 23 * P
    p["groups"] = [
        list(range(g, min(g + GB, p["NB"]))) for g in range(0, p["NB"], GB)
    ]
    # embedding segments: two halves per region, uniform 13-block capacity
    p["emb_segs"] = []
    for b0, b1_ in p["reg_blocks"]:
        p["emb_segs"].append((b0, min(b0 + 13, b1_)))
        p["emb_segs"].append((min(b0 + 13, b1_), b1_))
    p["NGT"] = G // P
    assert G % P == 0
    return p


def _wrap16(idx_flat):
    """int16 index list -> [128, n/16] wrapped in 16 partitions, replicated 8x."""
    return np.tile(idx_flat.reshape(-1, 16).T, (8, 1))


def _prep_host(x, edge_index, batch, cfg):
    """Integer/index preprocessing + per-core metadata. Returns (per_core, Cq, pl)."""
    pl = _plan(cfg)
    N, G = cfg["N"], cfg["G"]
    NPC, NB, NBP = pl["NPC"], pl["NB"], pl["NBP"]

    src = np.asarray(edge_index[0], np.int64)
    dst = np.asarray(edge_index[1], np.int64)
    batch = np.asarray(batch, np.int64)
    x = np.asarray(x, np.int64)

    deg_p1 = (np.bincount(dst, minlength=N) + 1).astype(np.float64)
    dinv = 1.0 / np.sqrt(deg_p1)  # global normalization coefs
    cnt = np.maximum(np.bincount(batch, minlength=G), 1).astype(np.float32)
    cnt_pt = cnt.reshape(pl["NGT"], P).T.copy()  # [P, NGT]

    # region-local row index of a global src node
    s_core = src // NPC
    s_off = src % NPC
    s_reg = np.minimum(s_off // RSIZE, NREG - 1)
    s_row = s_core * RSIZE + (s_off - s_reg * RSIZE)  # < 25600

    core_of = dst // NPC
    per_core_edge = []
    Cq = 1
    for k in range(NCORES):
        m = core_of == k
        d_k = dst[m] - k * NPC
        blk = d_k >> 7
        key = (blk * NREG + s_reg[m]).astype(np.int64)
        order = np.argsort(key, kind="stable")
        counts = np.bincount(key, minlength=NB * NREG)
        Cq = max(Cq, math.ceil(counts.max() / P))
        sloc = s_row[m].astype(np.int16)[order]
        dslot = (d_k & 127).astype(np.int16)[order]
        coef = (dinv[src[m]] * dinv[dst[m]]).astype(np.float32)[order]
        per_core_edge.append((sloc, dslot, coef, counts))

    cap = Cq * P
    per_core = []
    for k in range(NCORES):
        sloc, dslot, coef, counts = per_core_edge[k]
        src_pad = np.zeros((NB * NREG, cap), np.int16)
        dst_pad = np.full((NB * NREG, cap), -1, np.int16)
        coef_pad = np.zeros((NB * NREG, cap), np.float32)
        starts = np.concatenate([[0], np.cumsum(counts)])
        for cell in range(NB * NREG):
            c0, c1 = starts[cell], starts[cell + 1]
            n = c1 - c0
            if n:
                src_pad[cell, :n] = sloc[c0:c1]
                dst_pad[cell, :n] = dslot[c0:c1]
                coef_pad[cell, :n] = coef[c0:c1]

        # bias slot: one pad slot per block (in its region-3 cell, whose table
        # row BIAS_ROW holds b) -> gathers the b row; one-hot row = all ones
        # so every dst of the block receives +b[f].
        BIAS = pl["BIAS_ROW"]
        for b in range(NB):
            cell = b * NREG + (NREG - 1)
            n = counts[cell]
            assert n < cap, f"core {k} block {b}: no pad slot for bias"
            src_pad[cell, n] = BIAS
            dst_pad[cell, n] = -2  # marker: full row of ones

        # gather index stream per (group, region): concat cell streams
        idx_cols = []
        for blocks in pl["groups"]:
            for r in range(NREG):
                cells = [b * NREG + r for b in blocks]
                flat = src_pad[cells].reshape(-1)
                idx_cols.append(_wrap16(flat))
        edge_idx = np.concatenate(idx_cols, 1)  # [128, totcol] i16

        # host-built one-hot matrices, block-major, chunk order (r, cc):
        # oh[p, b*4Cq*128 + (r*Cq+cc)*128 + i] = coef of edge slot p of that
        # chunk if its dst slot == i (or 1.0 for the bias slot's full row).
        dst3 = dst_pad.reshape(NB, NREG * Cq, P)  # [b, chunk, p]
        coef3 = coef_pad.reshape(NB, NREG * Cq, P)
        iota = np.arange(P, dtype=np.int16)
        onehot = (dst3[:, :, :, None] == iota[None, None, None, :]).astype(
            np.float32
        )
        onehot *= coef3[:, :, :, None]
        onehot += (dst3[:, :, :, None] == -2).astype(np.float32)
        # -> [p, b, chunk, i] -> [128, NB*4Cq*128]
        oh = (
            onehot.transpose(2, 0, 1, 3)
            .reshape(P, NB * NREG * Cq * P)
            .astype(NP_FP8)
        )

        # degree (layout [p, c] = local node c*128+p), pad nodes -> deg+1 = 1
        dp = np.ones(NBP, np.float32)
        dp[: NPC] = deg_p1[k * NPC : (k + 1) * NPC].astype(np.float32)
        dp = dp.reshape(NB, P).T.copy()

        # pool metadata
        bl = batch[k * NPC : (k + 1) * NPC]
        gbase = int(bl[0])
        gspan = int(bl[-1]) - gbase + 1
        assert gspan <= 2 * P, f"core {k} graph span {gspan} > 256"
        blf = np.full(NBP, -1.0, np.float32)
        blf[: NPC] = (bl - gbase).astype(np.float32)
        bl0 = blf.reshape(NB, P).T.astype(np.float32)
        bl1 = (blf - P).reshape(NB, P).T.astype(np.float32)
        gidx = np.zeros((P, 2), np.int32)
        for h in range(2):
            v = gbase + h * P + np.arange(P)
            v = np.where(v < G, v, G + (v % 8))
            gidx[:, h] = v

        # embedding gather indices into the combined [288, F] bf16 table:
        # 8 segments of up-to-13 blocks, j-major within each segment
        xi = np.zeros((NBP, 3), np.int16)
        xi[: NPC] = x[k * NPC : (k + 1) * NPC].astype(np.int16)
        xi[:, 1] += 16
        xi[:, 2] += 32
        emb_cols = []
        for s0, s1 in pl["emb_segs"]:
            seg = np.zeros((3, 13 * P), np.int16)
            seg[:, : (s1 - s0) * P] = xi[s0 * P : s1 * P].T
            emb_cols.append(_wrap16(seg.reshape(-1)))
        emb_idx = np.concatenate(emb_cols, 1)  # [128, 8*3*13*8]

        per_core.append(
            dict(
                deg_p1=dp,
                bl0=bl0,
                bl1=bl1,
                gidx=gidx,
                cnt=cnt_pt,
                emb_idx=emb_idx,
                edge_idx=edge_idx,
                oh=oh,
            )
        )
    return per_core, Cq, pl


def _build(cfg, Cq, pl, totcol, nohc):
    """Build the SPMD Bass program (one NEFF for all 8 cores)."""
    N, G, F, NCLS = cfg["N"], cfg["G"], cfg["F"], cfg["NCLS"]
    NPC, NB, NGT = pl["NPC"], pl["NB"], pl["NGT"]
    groups, reg_blocks, TROWS = pl["groups"], pl["reg_blocks"], pl["TROWS"]
    pl_emb = pl["emb_segs"]
    CPB = NREG * Cq  # chunks per block

    nc = bacc.Bacc("TRN2", num_devices=NCORES, num_swdge_queues=4)
    RG = [list(range(NCORES))]

    # ---- I/O ----
    tabs = [
        nc.dram_tensor("shape_tab", [16, F], F32, kind="ExternalInput"),
        nc.dram_tensor("color_tab", [16, F], F32, kind="ExternalInput"),
        nc.dram_tensor("pos_tab", [256, F], F32, kind="ExternalInput"),
    ]
    W1d = nc.dram_tensor("W1", [F, F], F32, kind="ExternalInput")
    W2d = nc.dram_tensor("W2", [F, F], F32, kind="ExternalInput")
    b1d = nc.dram_tensor("b1", [1, F], F32, kind="ExternalInput")
    b2d = nc.dram_tensor("b2", [1, F], F32, kind="ExternalInput")
    Wld = nc.dram_tensor("Wlin", [F, NCLS], F32, kind="ExternalInput")
    bld = nc.dram_tensor("blin", [1, NCLS], F32, kind="ExternalInput")
    degd = nc.dram_tensor("deg_p1", [P, NB], F32, kind="ExternalInput")
    bl0d = nc.dram_tensor("bl0", [P, NB], F32, kind="ExternalInput")
    bl1d = nc.dram_tensor("bl1", [P, NB], F32, kind="ExternalInput")
    gixd = nc.dram_tensor("gidx", [P, 2], I32, kind="ExternalInput")
    cntd = nc.dram_tensor("cnt", [P, NGT], F32, kind="ExternalInput")
    eixd = nc.dram_tensor("emb_idx", [P, 8 * 3 * 13 * 8], I16, kind="ExternalInput")
    xixd = nc.dram_tensor("edge_idx", [P, totcol], I16, kind="ExternalInput")
    ohd = nc.dram_tensor("oh", [P, nohc], FP8, kind="ExternalInput")
    outd = nc.dram_tensor("out", [G, NCLS], F32, kind="ExternalOutput")

    with tile.TileContext(nc) as tc:
        import contextlib

        ctx = contextlib.ExitStack()
        persist = ctx.enter_context(tc.tile_pool(name="persist", bufs=1))
        dramp = ctx.enter_context(tc.tile_pool(name="dramp", bufs=1, space="DRAM"))
        xw_pool = ctx.enter_context(tc.tile_pool(name="xw", bufs=2, space="PSUM"))
        tp_pool = ctx.enter_context(tc.tile_pool(name="tp", bufs=1, space="PSUM"))
        acc_pool = ctx.enter_context(tc.tile_pool(name="acc", bufs=3, space="PSUM"))
        pacc_pool = ctx.enter_context(tc.tile_pool(name="pacc", bufs=1, space="PSUM"))
        sb_pool = ctx.enter_context(tc.tile_pool(name="work", bufs=4))
        hreg_pool = ctx.enter_context(tc.tile_pool(name="hreg", bufs=2))
        yreg_pool = ctx.enter_context(tc.tile_pool(name="yreg", bufs=2))
        msg_pool = ctx.enter_context(tc.tile_pool(name="msg", bufs=9))
        oh_pool = ctx.enter_context(tc.tile_pool(name="oh", bufs=3))
        ix_pool = ctx.enter_context(tc.tile_pool(name="ix", bufs=6))
        craw = ctx.enter_context(tc.tile_pool(name="craw", bufs=1))

        def T(shape, dt, space=None, addr_space="Local", name=None):
            pool = dramp if space == "DRAM" else persist
            return pool.tile(shape, dt, tag=name, name=name, addr_space=addr_space)

        # ---- internal DRAM ----
        y_slice = [
            [T([RSIZE, F], BF16, space="DRAM", name=f"ysl{c}_{r}") for r in range(NREG)]
            for c in range(2)
        ]
        y_reg = [
            [
                T([TROWS, F], BF16, space="DRAM", addr_space="Shared",
                  name=f"yreg{c}_{r}")
                for r in range(NREG)
            ]
            for c in range(2)
        ]
        dram_sums = T([G + 8, F], F32, space="DRAM", name="dram_sums")
        ar_sums = T([G + 8, F], F32, space="DRAM", addr_space="Shared",
                    name="ar_sums")

        # ---- persistent SBUF ----
        hmid = T([P, NB * F], BF16, name="hmid")  # conv1 output
        y_nm = T([P, NB * F], BF16, name="y_nm")  # xw/(deg+1) for self-loop

        # constants
        iota_i = craw.tile([P, P], I32, tag="iota_i", name="iota_i")
        nc.gpsimd.iota(iota_i[:], pattern=[[1, P]], base=0, channel_multiplier=0)
        iota_f = T([P, P], F32, name="iota_f")
        nc.vector.tensor_copy(iota_f[:], iota_i[:])
        id_f32 = T([P, P], F32, name="id_f32")
        make_identity(nc, id_f32[:])
        id_bf = T([P, P], BF16, name="id_bf")
        nc.vector.tensor_copy(id_bf[:], id_f32[:])
        ones_row = T([1, P], BF16, name="ones_row")
        nc.vector.memset(ones_row[:], 1.0)

        def load_cast(name, dram, shape, dt_in, dt_out):
            t = T(shape, dt_out, name=name)
            if dt_out == dt_in:
                nc.sync.dma_start(out=t[:], in_=dram[:])
            else:
                raw = craw.tile(shape, dt_in, tag=name + "_r", name=name + "_r")
                nc.sync.dma_start(out=raw[:], in_=dram[:])
                nc.vector.tensor_copy(t[:], raw[:])
            return t

        W1c = load_cast("W1", W1d, [F, F], F32, BF16)
        W2c = load_cast("W2", W2d, [F, F], F32, BF16)
        bc = [
            load_cast("b1", b1d, [1, F], F32, BF16),
            load_cast("b2", b2d, [1, F], F32, BF16),
        ]
        Wl_sb = load_cast("Wl", Wld, [F, NCLS], F32, BF16)
        bl_sb = load_cast("bl", bld, [1, NCLS], F32, BF16)
        bl0_sb = load_cast("bl0", bl0d, [P, NB], F32, F32)
        bl1_sb = load_cast("bl1", bl1d, [P, NB], F32, F32)
        cnt_sb = load_cast("cnt", cntd, [P, NGT], F32, F32)
        gix_sb = load_cast("gix", gixd, [P, 2], I32, I32)
        eix_sb = load_cast("eix", eixd, [P, 8 * 3 * 13 * 8], I16, I16)

        deg_sb = craw.tile([P, NB], F32, tag="deg_sb", name="deg_sb")
        nc.sync.dma_start(out=deg_sb[:], in_=degd[:])
        rdeg = T([P, NB], F32, name="rdeg")  # 1/(deg+1)
        nc.vector.reciprocal(rdeg[:], deg_sb[:])

        # combined bf16 embedding table in DRAM: rows 0-15 shape, 16-31 color,
        # 32-287 pos
        tab_bf = T([288, F], BF16, space="DRAM", name="tab_bf")
        for j, (tab, rows, r0) in enumerate(
            [(tabs[0], 16, 0), (tabs[1], 16, 16), (tabs[2], 128, 32), (tabs[2], 128, 160)]
        ):
            traw = craw.tile([rows, F], F32, tag=f"tab{j}", name=f"tab{j}")
            nc.sync.dma_start(out=traw[:], in_=tab[r0 - 32 : r0 - 32 + rows, :] if j >= 2 else tab[:rows, :])
            tbf = craw.tile([rows, F], BF16, tag=f"tabb{j}", name=f"tabb{j}")
            nc.vector.tensor_copy(tbf[:], traw[:])
            nc.sync.dma_start(out=tab_bf[r0 : r0 + rows, :], in_=tbf[:])

        # zero dram_sums (pool scatter target) early
        zsb = craw.tile([P, 512], F32, tag="zsb", name="zsb")
        nc.vector.memset(zsb[:], 0.0)
        nrow = G + 8
        r_ = 0
        while r_ < nrow:
            take = min(512, ((nrow - r_) // P) * P)
            pp = P
            if take == 0:
                take = nrow - r_
                pp = take
            nc.sync.dma_start(
                out=dram_sums[r_ : r_ + take, :].rearrange("(c p) f -> p c f", p=pp),
                in_=zsb[:pp, : take * F // pp].rearrange("p (c f) -> p c f", f=F),
            )
            r_ += take

        # ---------------- conv phases ----------------
        def xw_tile(t, hsrc, Wc_, yreg_sb, yoff):
            """y_raw = h @ W for block t -> yreg_sb col yoff; y_nm = y_raw/(deg+1)."""
            tp = tp_pool.tile([P, P], BF16, tag="tp")
            nc.tensor.transpose(tp[:], hsrc, id_bf[:])
            hT = sb_pool.tile([P, P], BF16, tag="hT")
            nc.scalar.copy(hT[:], tp[:])
            xwp = xw_pool.tile([P, F], F32, tag="xw")
            nc.tensor.matmul(xwp[:], lhsT=hT[:], rhs=Wc_[:], start=True, stop=True)
            nc.scalar.copy(yreg_sb[:, yoff * F : (yoff + 1) * F], xwp[:])
            nc.scalar.activation(
                y_nm[:, t * F : (t + 1) * F],
                xwp[:],
                mybir.ActivationFunctionType.Copy,
                scale=rdeg[:, t : t + 1],
            )

        pacc = [
            pacc_pool.tile([P, P], F32, tag=f"pacc{h}", name=f"pacc{h}")
            for h in range(2)
        ]
        bls = [bl0_sb, bl1_sb]

        def finish_region(conv, r, yreg_sb):
            """y DMA + (bias row) + AllGather for region r of this conv."""
            b0, b1_ = reg_blocks[r]
            nblk = b1_ - b0
            nc.sync.dma_start(
                out=y_slice[conv][r][: nblk * P, :].rearrange(
                    "(c p) f -> p c f", p=P
                ),
                in_=yreg_sb[:, : nblk * F].rearrange("p (c f) -> p c f", f=F),
            )
            if r == NREG - 1:
                # bias row rides in region 3's unused tail (table row 2944)
                nc.sync.dma_start(
                    out=y_slice[conv][r][23 * P : 23 * P + 1, :],
                    in_=bc[conv][:],
                )
            nc.gpsimd.collective_compute(
                "AllGather",
                mybir.AluOpType.bypass,
                replica_groups=RG,
                ins=[y_slice[conv][r][:]],
                outs=[y_reg[conv][r][:]],
            )

        # ---- conv1 xw phase: embedding gathers (8 segments) + xw + AG1 ----
        ES = 3 * 13 * P  # idx per embed segment
        yreg_sb = None
        for s, (s0, s1) in enumerate(pl_emb):
            nt = s1 - s0
            hraw = hreg_pool.tile([P, 3, 13, F], BF16, tag="hraw")
            nc.gpsimd.dma_gather(
                out_ap=hraw[:].rearrange("p a b f -> p (a b) f"),
                in_ap=tab_bf[:, :],
                idxs_ap=eix_sb[:, s * ES // 16 : (s + 1) * ES // 16],
                num_idxs=ES,
                num_idxs_reg=ES,
                elem_size=F,
                single_packet=False,
                queue_num=s % 4,
            )
            h13 = hreg_pool.tile([P, 13, F], BF16, tag="h13")
            nc.vector.tensor_tensor(
                out=h13[:, :nt, :], in0=hraw[:, 0, :nt, :], in1=hraw[:, 1, :nt, :],
                op=mybir.AluOpType.add,
            )
            nc.vector.tensor_tensor(
                out=h13[:, :nt, :], in0=h13[:, :nt, :], in1=hraw[:, 2, :nt, :],
                op=mybir.AluOpType.add,
            )
            r = s // 2
            b0, b1_ = reg_blocks[r]
            if s % 2 == 0:
                yreg_sb = yreg_pool.tile([P, 25 * F], BF16, tag="yreg")
            for t in range(s0, s1):
                xw_tile(t, h13[:, t - s0, :], W1c, yreg_sb, t - b0)
            if s % 2 == 1:
                finish_region(0, r, yreg_sb)

        # ---- scatter phase (conv1 fuses conv2's xw+AG; conv2 fuses pool) ----
        for conv in range(2):
            col_off = 0
            yreg2 = None
            for gi, blocks in enumerate(groups):
                nblk = len(blocks)
                nch_q = nblk * Cq
                nidx = nch_q * P
                msgs = []
                for r in range(NREG):
                    ixt = ix_pool.tile([P, GB * Cq * 8], I16, tag="ix")
                    nc.sync.dma_start(
                        out=ixt[:, : nidx // 16],
                        in_=xixd[:, col_off : col_off + nidx // 16],
                    )
                    col_off += nidx // 16
                    msg = msg_pool.tile([P, GB * Cq, F], BF16, tag="msg")
                    nc.gpsimd.dma_gather(
                        out_ap=msg[:, :nch_q, :],
                        in_ap=y_reg[conv][r][:, :],
                        idxs_ap=ixt[:, : nidx // 16],
                        num_idxs=nidx,
                        num_idxs_reg=nidx,
                        elem_size=F,
                        single_packet=False,
                        queue_num=r,
                    )
                    msgs.append(msg)
                for bi, b in enumerate(blocks):
                    # oh loads ride the scalar engine's HWDGE ring so they
                    # don't serialize behind idx/y DMAs on the sync ring
                    ohs = oh_pool.tile([P, CPB * P], FP8, tag="oh")
                    nc.scalar.dma_start(
                        out=ohs[:],
                        in_=ohd[:, b * CPB * P : (b + 1) * CPB * P],
                    )
                    acc = acc_pool.tile([P, P], F32, tag="acc")
                    nc.tensor.matmul(
                        acc[:],
                        lhsT=id_bf[:],
                        rhs=y_nm[:, b * F : (b + 1) * F],
                        start=True,
                        stop=False,
                    )
                    for r in range(NREG):
                        for cc in range(Cq):
                            j = r * Cq + cc
                            nc.tensor.matmul(
                                acc[:],
                                lhsT=ohs[:, j * P : (j + 1) * P],
                                rhs=msgs[r][:, bi * Cq + cc, :],
                                start=False,
                                stop=(j == CPB - 1),
                            )
                    if conv == 0:
                        nc.scalar.activation(
                            hmid[:, b * F : (b + 1) * F],
                            acc[:],
                            mybir.ActivationFunctionType.Relu,
                        )
                        # conv2's xw for this block, fused into conv1 scatter
                        r2 = min(b // 25, NREG - 1)
                        rb0, rb1 = reg_blocks[r2]
                        if b == rb0:
                            yreg2 = yreg_pool.tile([P, 25 * F], BF16, tag="yreg")
                        xw_tile(b, hmid[:, b * F : (b + 1) * F], W2c, yreg2, b - rb0)
                        if b == rb1 - 1:
                            finish_region(1, r2, yreg2)
                    else:
                        htmp = sb_pool.tile([P, P], BF16, tag="htmp")
                        nc.scalar.activation(
                            htmp[:], acc[:], mybir.ActivationFunctionType.Relu
                        )
                        for h in range(2):
                            oht = sb_pool.tile([P, P], BF16, tag="pooloh")
                            nc.vector.tensor_tensor(
                                out=oht[:],
                                in0=iota_f[:],
                                in1=bls[h][:, b : b + 1].to_broadcast([P, P]),
                                op=mybir.AluOpType.is_equal,
                            )
                            nc.tensor.matmul(
                                pacc[h][:],
                                lhsT=oht[:],
                                rhs=htmp[:],
                                start=(b == 0),
                                stop=(b == NB - 1),
                            )

        # ---------------- global mean pool + linear ----------------
        for h in range(2):
            se = sb_pool.tile([P, P], F32, tag="ep")
            nc.vector.tensor_copy(se[:], pacc[h][:])
            nc.gpsimd.indirect_dma_start(
                out=dram_sums[:],
                out_offset=bass.IndirectOffsetOnAxis(ap=gix_sb[:, h : h + 1], axis=0),
                in_=se[:],
                in_offset=None,
            )
        nc.gpsimd.collective_compute(
            "AllReduce",
            mybir.AluOpType.add,
            replica_groups=RG,
            ins=[dram_sums[:]],
            outs=[ar_sums[:]],
        )
        recip = T([P, NGT], F32, name="recip")
        nc.vector.reciprocal(recip[:], cnt_sb[:])
        for t in range(NGT):
            art = sb_pool.tile([P, P], F32, tag="art")
            nc.sync.dma_start(out=art[:], in_=ar_sums[t * P : (t + 1) * P, :])
            pooled = sb_pool.tile([P, P], BF16, tag="ep")
            nc.vector.tensor_tensor(
                out=pooled[:],
                in0=art[:],
                in1=recip[:, t : t + 1].to_broadcast([P, P]),
                op=mybir.AluOpType.mult,
            )
            ptp = tp_pool.tile([P, P], BF16, tag="tp")
            nc.tensor.transpose(ptp[:], pooled[:], id_bf[:])
            pooledT = sb_pool.tile([P, P], BF16, tag="hT2")
            nc.scalar.copy(pooledT[:], ptp[:])
            op = xw_pool.tile([P, NCLS], F32, tag="xw")
            nc.tensor.matmul(op[:], lhsT=pooledT[:], rhs=Wl_sb[:], start=True, stop=False)
            nc.tensor.matmul(op[:], lhsT=ones_row[:], rhs=bl_sb[:], start=False, stop=True)
            oute = sb_pool.tile([P, NCLS], F32, tag="oute")
            nc.vector.tensor_copy(oute[:], op[:])
            nc.sync.dma_start(out=outd[t * P : (t + 1) * P, :], in_=oute[:])

        ctx.close()
    nc.compile()
    return nc


_CACHE = {}


def _get_nc(cfg, Cq, pl, totcol, nohc):
    key = (tuple(sorted(cfg.items())), Cq, totcol, nohc)
    if key not in _CACHE:
        _CACHE[key] = _build(cfg, Cq, pl, totcol, nohc)
    return _CACHE[key]


def run(inputs, cfg, trace=False):
    x = np.asarray(inputs["x"])
    per_core, Cq, pl = _prep_host(
        x, np.asarray(inputs["edge_index"]), np.asarray(inputs["batch"]), cfg
    )
    totcol = per_core[0]["edge_idx"].shape[1]
    nohc = per_core[0]["oh"].shape[1]
    nc = _get_nc(cfg, Cq, pl, totcol, nohc)

    shared = dict(
        shape_tab=np.asarray(inputs["shape_tab"], np.float32),
        color_tab=np.asarray(inputs["color_tab"], np.float32),
        pos_tab=np.asarray(inputs["pos_tab"], np.float32),
        W1=np.asarray(inputs["W1"], np.float32),
        W2=np.asarray(inputs["W2"], np.float32),
        b1=np.asarray(inputs["b1"], np.float32).reshape(1, -1),
        b2=np.asarray(inputs["b2"], np.float32).reshape(1, -1),
        Wlin=np.asarray(inputs["Wlin"], np.float32),
        blin=np.asarray(inputs["blin"], np.float32).reshape(1, -1),
    )
    in_maps = [{**shared, **per_core[k]} for k in range(NCORES)]
    res = run_bass_kernel_spmd(nc, in_maps, list(range(NCORES)), trace=trace)
    out = np.asarray(res.results[0]["out"], np.float32)
    return out, res


def kernel(**inputs) -> np.ndarray:
    out, _ = run(inputs, CFG)
    return out
